# revision 32
# baseline (speedup 1.0000x reference)
"""Trainium2 Bass kernel for nn_Block_80041010528755 (spiking transformer block).

Math structure (see reference):
  q = spike(LN(x@q_w) >= 2), k/v likewise (binary {0,1})
  attn has NO softmax -> (q@k^T)@v == q@(k^T@v): per-head 64x64 kv matrix,
  exact in bf16/fp32 because spikes are binary and sums are small integers.
  y2 = spike(LN(yspike@proj_w + pb) >= 2); x' = x + y2
  m1 = spike(LN(x'@fc1_w + b1) >= 2); m2 = spike(LN(m1@fc2_w + b2) >= 2)
  out = x' + m2

Precision: fp32-input matmuls (q/k/v from x, fc1 from x') use 3-product
bf16 hi/lo splits (x_hi@W_hi + x_lo@W_hi + x_hi@W_lo, ~2^-16 rel);
binary-input matmuls (proj, fc2) use 2 products (S@W_hi + S@W_lo).
All accumulate in fp32 PSUM.

Sharding: 8-way token-parallel, 512 tokens/core (half a batch). k/v are
computed over the core's full 1024-token batch (duplicated within the
core pair) so attention needs no collectives.
"""

import os
import sys

for _p in ("/root/.axon_site/_ro/trn_rl_repo", "/opt/trn_rl_repo"):
    if os.path.isdir(_p) and _p not in sys.path:
        sys.path.append(_p)

import numpy as np
import ml_dtypes

import concourse.bass as bass
import concourse.bacc as bacc
import concourse.tile as tile
import concourse.mybir as mybir
from concourse.bass import ts
from concourse.bass_utils import run_bass_kernel_spmd

F32 = mybir.dt.float32
BF16 = mybir.dt.bfloat16
OP = mybir.AluOpType

B, L, D = 4, 1024, 1024
HID = 4096
H, HD = 16, 64
NCORES = 8
T = 512          # own tokens per core
TB = 1024        # batch tokens per core (own + partner half)
P = 128
LN_EPS = 1e-5
THETA = 2.0      # LN-spike threshold: TAU*v_th = 2*1
ATTN_THETA = 1.0  # attn spike: y >= TAU*0.5

# module-global stash for timing info from the last kernel() call
last_run_info = {}


def _split_hi_lo(a32):
    hi = a32.astype(ml_dtypes.bfloat16)
    lo = (a32 - hi.astype(np.float32)).astype(ml_dtypes.bfloat16)
    return np.ascontiguousarray(hi), np.ascontiguousarray(lo)


def _bcast_ap(dram_ap, parts=P):
    """[D] dram tensor viewed as [parts, D] with 0-stride partitions."""
    return bass.AP(tensor=dram_ap.tensor, offset=dram_ap.offset,
                   ap=[[0, parts]] + list(dram_ap.ap))


def build_program(cfg, debug_outputs=False):
    """cfg: dict with has_bias flags + g-sign modes per LN stage."""
    nc = bacc.Bacc("TRN2", target_bir_lowering=False, debug=False)

    # ---- DRAM tensors ----
    xT_hi = nc.dram_tensor("xT_hi", [D, TB], BF16, kind="ExternalInput")
    xT_lo = nc.dram_tensor("xT_lo", [D, TB], BF16, kind="ExternalInput")
    x_tok = nc.dram_tensor("x_tok", [T, D], F32, kind="ExternalInput")

    w_names = {}
    for nm, (din, dout) in (("qw", (D, D)), ("kw", (D, D)), ("vw", (D, D)),
                            ("pw", (D, D)), ("f1", (D, HID)), ("f2", (HID, D))):
        for h in ("hi", "lo"):
            w_names[f"{nm}_{h}"] = nc.dram_tensor(
                f"{nm}_{h}", [din, dout], BF16, kind="ExternalInput")

    thr_names = {}
    for nm, dd in (("tq", D), ("tk", D), ("tv", D), ("tp", D),
                   ("t1", HID), ("t2", D)):
        thr_names[nm] = nc.dram_tensor(nm, [dd], F32, kind="ExternalInput")

    ident_in = nc.dram_tensor("ident", [P, P], BF16, kind="ExternalInput")
    ws1_hi = nc.dram_tensor("ws1_hi", [D], BF16, kind="ExternalInput")
    ws1_lo = nc.dram_tensor("ws1_lo", [D], BF16, kind="ExternalInput")

    bias_names = {}
    for nm, dd in (("bp", D), ("b1", HID), ("b2", D)):
        if cfg[f"has_{nm}"]:
            bias_names[nm] = nc.dram_tensor(nm, [dd], F32, kind="ExternalInput")

    out_dram = nc.dram_tensor("out", [T, D], F32, kind="ExternalOutput")

    dbg = {}
    if debug_outputs:
        for nm, shp, dt in (("d_qsT", [D, T], BF16), ("d_ks", [TB, D], BF16),
                            ("d_vs", [TB, D], BF16), ("d_ysT", [D, T], BF16),
                            ("d_y2", [T, D], BF16), ("d_m1T", [HID, T], BF16),
                            ("d_z1T", [HID, T], F32)):
            dbg[nm] = nc.dram_tensor(nm, shp, dt, kind="ExternalOutput")

    # weight dram views [p, kc, dout]
    wv = {k: v.ap().rearrange("(kc p) f -> p kc f", p=P)
          for k, v in w_names.items()}

    def dbg_copy(dram, sb, fm=False):
        pat = "(c p) t -> p c t" if fm else "(c p) f -> p c f"
        dv = dram.ap().rearrange(pat, p=P)
        for c in range(sb.shape[1]):
            nc.sync.dma_start(dv[:, c, :], sb[:, c, :])

    with tile.TileContext(nc) as tc:
        with tc.tile_pool(name="psum", bufs=6, space="PSUM") as psum, \
             tc.tile_pool(name="stats", bufs=6) as stats, \
             tc.tile_pool(name="thrp", bufs=3) as thrp, \
             tc.tile_pool(name="consts", bufs=1) as consts, \
             tc.tile_pool(name="resid", bufs=1) as resid:

            eps_t = consts.tile([P, 1], F32)
            nc.vector.memset(eps_t, LN_EPS)
            ident = consts.tile([P, P], BF16, tag="ident")
            nc.sync.dma_start(ident, ident_in.ap())
            ones0 = consts.tile([P, 1], F32, tag="ones0")
            nc.vector.memset(ones0, 1.0)
            ones_r = consts.tile([P, 1], mybir.dt.float32r, tag="ones_r")
            nc.vector.tensor_copy(ones_r, ones0)

            # round-robin 128x128 DMA transposes over the two HWDGE queues
            _tp_state = [0]

            def tpose(dst, src):
                eng = (nc.sync, nc.sync)[_tp_state[0] % 2]
                _tp_state[0] += 1
                eng.dma_start(out=dst, in_=src, transpose=True)

            def load_bcast(name, dd, pool):
                t = pool.tile([P, dd], F32, tag=f"bc_{name}", name=f"bc_{name}")
                nc.gpsimd.dma_start(t, _bcast_ap(thr_names[name].ap()
                                                 if name in thr_names
                                                 else bias_names[name].ap()))
                return t

            # ---------- LN + spike helper ----------
            def ln_spike(z_chunks, thr_b, mode, out_fn, stat_tag):
                """z_chunks: APs [128, 512] covering the feature dim for one
                token block. thr_b: [128, d_total] bcast of (theta-b)/g.
                Emits spike = cmp(z, mean + t*std) per chunk via out_fn."""
                nchunks = len(z_chunks)
                st = stats.tile([P, nchunks, 6], F32, tag=f"st_{stat_tag}",
                                name=f"st_{stat_tag}")
                for j, zc in enumerate(z_chunks):
                    nc.vector.bn_stats(st[:, j], zc)
                mv = stats.tile([P, 2], F32, tag=f"mv_{stat_tag}",
                                name=f"mv_{stat_tag}")
                nc.vector.bn_aggr(mv, st)
                std = stats.tile([P, 1], F32, tag=f"sd_{stat_tag}",
                                 name=f"sd_{stat_tag}")
                nc.scalar.activation(out=std, in_=mv[:, 1:2],
                                     func=mybir.ActivationFunctionType.Sqrt,
                                     bias=eps_t, scale=1.0)
                cmp = OP.is_ge if mode == "pos" else OP.is_le
                for j, zc in enumerate(z_chunks):
                    thr = thrp.tile([P, 512], F32, tag="thr", name=f"th_{stat_tag}_{j}")
                    nc.vector.tensor_scalar(out=thr, in0=thr_b[:, ts(j, 512)],
                                            scalar1=std, scalar2=mv[:, 0:1],
                                            op0=OP.mult, op1=OP.add)
                    out_fn(j, zc, thr, cmp)

            xnew = resid.tile([P, T // P, D], F32, tag="xnew")

            with tc.tile_pool(name="xtp", bufs=1) as xtp:
                xt = xtp.tile([P, T // P, D], F32, tag="xt")
                # off the sync queue: only needed at proj time
                nc.gpsimd.dma_start(xt, x_tok.ap().rearrange("(c p) f -> p c f",
                                                             p=P))

                with tc.tile_pool(name="sp3", bufs=1) as sp3:
                    ysT = sp3.tile([P, D // P, T], BF16, tag="ysT")
                    y2 = sp3.tile([P, T // P, D], BF16, tag="y2")

                    with tc.tile_pool(name="sp12", bufs=1) as sp12:
                        kS = sp12.tile([P, TB // P, D], BF16, tag="kS")
                        vS = sp12.tile([P, TB // P, D], BF16, tag="vS")
                        qTS = sp12.tile([P, D // P, T], BF16, tag="qTS")

                        # ======== stage 1+2: q (own), k, v (batch) =========
                        with tc.tile_pool(name="xTp", bufs=1) as xTpool, \
                             tc.tile_pool(name="tqkv", bufs=1) as tpool, \
                             tc.tile_pool(name="qsc", bufs=3) as qscp, \
                             tc.tile_pool(name="wqkvh", bufs=2) as wqkvh, \
                             tc.tile_pool(name="wqkvl", bufs=1) as wqkvl:
                            xTh = xTpool.tile([P, D // P, TB], BF16, tag="xTh")
                            xTl = xTpool.tile([P, D // P, TB], BF16, tag="xTl")
                            xThd = xT_hi.ap().rearrange("(c p) t -> p c t", p=P)
                            xTld = xT_lo.ap().rearrange("(c p) t -> p c t", p=P)
                            # own-token halves first (q only needs these),
                            # partner halves after the q weights
                            for kk in range(D // P):
                                nc.sync.dma_start(xTh[:, kk, 0:T], xThd[:, kk, 0:T])
                                nc.sync.dma_start(xTl[:, kk, 0:T], xTld[:, kk, 0:T])
                            # bcast thresholds off the sync queue
                            tq_b = load_bcast("tq", D, tpool)
                            tk_b = load_bcast("tk", D, tpool)
                            tv_b = load_bcast("tv", D, tpool)

                            for nm, spk, thr_b, ntt, mode in (
                                    ("qw", None, tq_b, T // P, cfg["mode_q"]),
                                    ("kw", kS, tk_b, TB // P, cfg["mode_k"]),
                                    ("vw", vS, tv_b, TB // P, cfg["mode_v"])):
                                if True:
                                    whi = wqkvh.tile([P, D // P, D], BF16,
                                                     tag="wqkv_hi",
                                                     name=f"{nm}_hi_t")
                                    wlo = wqkvl.tile([P, D // P, D], BF16,
                                                     tag="wqkv_lo",
                                                     name=f"{nm}_lo_t")
                                    for kk in range(D // P):
                                        nc.sync.dma_start(whi[:, kk],
                                                          wv[f"{nm}_hi"][:, kk])
                                        nc.sync.dma_start(wlo[:, kk],
                                                          wv[f"{nm}_lo"][:, kk])
                                    if nm == "qw":
                                        # partner xT halves: needed from k on
                                        for kk in range(D // P):
                                            nc.sync.dma_start(
                                                xTh[:, kk, T:TB],
                                                xThd[:, kk, T:TB])
                                            nc.sync.dma_start(
                                                xTl[:, kk, T:TB],
                                                xTld[:, kk, T:TB])
                                    for tt in range(ntt):
                                        pss = []
                                        for n in range(D // 512):
                                            ps = psum.tile([P, 512], F32, tag="mm",
                                                           name=f"ps_{nm}_{tt}_{n}")
                                            first = True
                                            for xa, wa in ((xTh, whi), (xTl, whi),
                                                           (xTh, wlo)):
                                                for kk in range(D // P):
                                                    nc.tensor.matmul(
                                                        ps, xa[:, kk, ts(tt, P)],
                                                        wa[:, kk, ts(n, 512)],
                                                        start=first,
                                                        stop=(xa is xTh and
                                                              wa is wlo and
                                                              kk == D // P - 1))
                                                    first = False
                                            pss.append(ps)

                                        if spk is None:
                                            # q: emit to scratch, transpose to
                                            # feature-major immediately
                                            def emit(j, zc, thc, cmp, tt=tt):
                                                qc = qscp.tile(
                                                    [P, 512], BF16, tag="qc",
                                                    name=f"qc_{tt}_{j}")
                                                nc.vector.tensor_tensor(
                                                    out=qc, in0=zc, in1=thc,
                                                    op=cmp)
                                                for j2 in range(4):
                                                    fcx = j * 4 + j2
                                                    tpose(qTS[:, fcx, ts(tt, P)],
                                                          qc[:, ts(j2, P)])
                                        else:
                                            def emit(j, zc, thc, cmp, spk=spk,
                                                     tt=tt):
                                                nc.vector.tensor_tensor(
                                                    out=spk[:, tt, ts(j, 512)],
                                                    in0=zc, in1=thc, op=cmp)
                                        ln_spike(pss, thr_b, mode, emit, "qkv")

                        if debug_outputs:
                            dbg_copy(dbg["d_qsT"], qTS, fm=True)
                            dbg_copy(dbg["d_ks"], kS)
                            dbg_copy(dbg["d_vs"], vS)

                        # ======== stage 3: kv + y + attn spike =============
                        with tc.tile_pool(name="attn", bufs=4) as apool:
                            for hp in range(D // P):   # 8 head pairs
                                pkv = psum.tile([P, P], F32, tag="mm",
                                                name=f"pkv_{hp}")
                                for tt in range(TB // P):
                                    nc.tensor.matmul(
                                        pkv, kS[:, tt, ts(hp, P)],
                                        vS[:, tt, ts(hp, P)],
                                        start=(tt == 0), stop=(tt == TB // P - 1))
                                kvd = apool.tile([P, P], F32, tag="kvd",
                                                 name=f"kvd_{hp}")
                                nc.vector.memset(kvd, 0.0)
                                nc.vector.tensor_scalar_mul(
                                    kvd[0:HD, 0:HD], pkv[0:HD, 0:HD], 0.125)
                                nc.vector.tensor_scalar_mul(
                                    kvd[HD:P, HD:P], pkv[HD:P, HD:P], 0.125)
                                kvh = apool.tile([P, P], BF16, tag="kvh",
                                                 name=f"kvh_{hp}")
                                nc.vector.tensor_copy(kvh, kvd)
                                kvhf = apool.tile([P, P], F32, tag="kvhf",
                                                  name=f"kvhf_{hp}")
                                nc.vector.tensor_copy(kvhf, kvh)
                                kvl = apool.tile([P, P], BF16, tag="kvl",
                                                 name=f"kvl_{hp}")
                                nc.vector.tensor_tensor(out=kvl, in0=kvd,
                                                        in1=kvhf, op=OP.subtract)
                                py = psum.tile([P, T], F32, tag="mm",
                                               name=f"py_{hp}")
                                nc.tensor.matmul(py, kvh, qTS[:, hp, :],
                                                 start=True, stop=False)
                                nc.tensor.matmul(py, kvl, qTS[:, hp, :],
                                                 start=False, stop=True)
                                nc.vector.tensor_scalar(out=ysT[:, hp, :],
                                                        in0=py,
                                                        scalar1=ATTN_THETA,
                                                        scalar2=None,
                                                        op0=OP.is_ge)
                    # sp12 closed: qS/kS/vS/qTS freed

                    if debug_outputs:
                        dbg_copy(dbg["d_ysT"], ysT, fm=True)

                    # ======== stage 4: proj + LN + spike, residual =========
                    with tc.tile_pool(name="wp", bufs=1) as wpool, \
                         tc.tile_pool(name="tproj", bufs=1) as tpool, \
                         tc.tile_pool(name="zproj", bufs=4) as zpool:
                        tp_b = load_bcast("tp", D, tpool)
                        bp_b = load_bcast("bp", D, tpool) if cfg["has_bp"] else None
                        pwh = wpool.tile([P, D // P, D], BF16, tag="w_pw_hi")
                        pwl = wpool.tile([P, D // P, D], BF16, tag="w_pw_lo")
                        nc.sync.dma_start(pwh, wv["pw_hi"])
                        nc.sync.dma_start(pwl, wv["pw_lo"])
                        for tt in range(T // P):
                            zrefs = []
                            for n in range(D // 512):
                                ps = psum.tile([P, 512], F32, tag="mm",
                                               name=f"ps_pr_{tt}_{n}")
                                first = True
                                for wa in (pwh, pwl):
                                    for kk in range(D // P):
                                        nc.tensor.matmul(
                                            ps, ysT[:, kk, ts(tt, P)],
                                            wa[:, kk, ts(n, 512)],
                                            start=first,
                                            stop=(wa is pwl and kk == D // P - 1))
                                        first = False
                                if bp_b is not None:
                                    zc = zpool.tile([P, 512], F32, tag="zproj",
                                                    name=f"zpr_{tt}_{n}")
                                    nc.vector.tensor_tensor(
                                        out=zc, in0=ps,
                                        in1=bp_b[:, ts(n, 512)], op=OP.add)
                                    zrefs.append(zc)
                                else:
                                    zrefs.append(ps)

                            def emit(j, zc, thc, cmp, tt=tt):
                                nc.vector.tensor_tensor(
                                    out=y2[:, tt, ts(j, 512)],
                                    in0=zc, in1=thc, op=cmp)
                            ln_spike(zrefs, tp_b, cfg["mode_p"], emit, "proj")
                            nc.vector.tensor_tensor(out=xnew[:, tt, :],
                                                    in0=xt[:, tt, :],
                                                    in1=y2[:, tt, :], op=OP.add)

                    if debug_outputs:
                        dbg_copy(dbg["d_y2"], y2)
                # sp3 closed: ysT, y2 freed
            # xtp closed: xt freed

            # ============ stage 5+6+7 ======================================
            with tc.tile_pool(name="sp6", bufs=1) as sp6:
                m1T = sp6.tile([P, HID // P, T], BF16, tag="m1T")

                with tc.tile_pool(name="sp5", bufs=1) as sp5:
                    xnT_h = sp5.tile([P, D // P, T], BF16, tag="xnT_h")
                    xnT_l = sp5.tile([P, D // P, T], BF16, tag="xnT_l")
                    # ---- stage 5: split xnew + PE transpose ----
                    with tc.tile_pool(name="xsplit", bufs=3) as xsp:
                        for tt in range(T // P):
                            xh = xsp.tile([P, D], BF16, tag="xh", name=f"xh_{tt}")
                            xl = xsp.tile([P, D], BF16, tag="xl", name=f"xl_{tt}")
                            xhf = xsp.tile([P, D], F32, tag="xhf", name=f"xhf_{tt}")
                            nc.vector.tensor_copy(xh, xnew[:, tt, :])
                            nc.vector.tensor_copy(xhf, xh)
                            nc.vector.tensor_tensor(out=xl, in0=xnew[:, tt, :],
                                                    in1=xhf, op=OP.subtract)
                            for fc in range(D // P):
                                for src, dst in ((xh, xnT_h), (xl, xnT_l)):
                                    pt = psum.tile([P, P], BF16, tag="mm",
                                                   name=f"pt_{tt}_{fc}")
                                    nc.tensor.transpose(pt, src[:, ts(fc, P)],
                                                        ident)
                                    nc.vector.tensor_copy(
                                        dst[:, fc, ts(tt, P)], pt)

                    # ---- stage 6: fc1, FEATURE-major so m1 spikes land
                    #      directly in fc2's lhsT layout (no transposes).
                    #      LN stats via fp32r ones-matmul reductions. ----
                    F32R = mybir.dt.float32r
                    NMC = HID // P   # 32 dout chunks
                    with tc.tile_pool(name="z1p", bufs=1) as z1pool, \
                         tc.tile_pool(name="tfc1", bufs=1) as tpool, \
                         tc.tile_pool(name="wf1", bufs=3) as wpool, \
                         tc.tile_pool(name="psred", bufs=1, space="PSUM") as psr, \
                         tc.tile_pool(name="fc1ln", bufs=4) as lp:
                        # t1 / b1 as per-partition [128, 32] (feature-major)
                        t1_fm = tpool.tile([P, NMC], F32, tag="t1_fm")
                        nc.sync.dma_start(
                            t1_fm, thr_names["t1"].ap().rearrange(
                                "(c p) -> p c", p=P))
                        b1_fm = None
                        if cfg["has_b1"]:
                            b1_fm = tpool.tile([P, NMC], F32, tag="b1_fm")
                            nc.sync.dma_start(
                                b1_fm, bias_names["b1"].ap().rearrange(
                                    "(c p) -> p c", p=P))
                        z1T = z1pool.tile([P, NMC, T], F32, tag="z1T")
                        pr_sum = psr.tile([1, T], F32, tag="pr_sum")
                        pr_sq = psr.tile([1, T], F32, tag="pr_sq")
                        cmp1 = OP.is_ge if cfg["mode_1"] == "pos" else OP.is_le

                        # mean*HID = xn @ rowsum(fc1_w) (+ sum(b1), host-folded)
                        wsh = tpool.tile([P, D // P], BF16, tag="ws1h")
                        wsl = tpool.tile([P, D // P], BF16, tag="ws1l")
                        nc.sync.dma_start(wsh, ws1_hi.ap().rearrange(
                            "(c p) -> p c", p=P))
                        nc.sync.dma_start(wsl, ws1_lo.ap().rearrange(
                            "(c p) -> p c", p=P))
                        # two token halves: half 0's LN-apply overlaps
                        # half 1's matmuls (fc1 weights streamed twice)
                        T2 = T // 2
                        for hf in range(2):
                            hsl = bass.ds(hf * T2, T2)
                            first = True
                            for xa, wa in ((xnT_h, wsh), (xnT_l, wsh),
                                           (xnT_h, wsl)):
                                for kk in range(D // P):
                                    nc.tensor.matmul(
                                        pr_sum[:, hsl], wa[:, kk:kk + 1],
                                        xa[:, kk, hsl],
                                        start=first,
                                        stop=(xa is xnT_h and wa is wsl and
                                              kk == D // P - 1))
                                    first = False

                            for mc in range(NMC):
                                wh = wpool.tile([P, D // P, P], BF16, tag="f1h",
                                                name=f"f1h_{hf}_{mc}")
                                wl = wpool.tile([P, D // P, P], BF16, tag="f1l",
                                                name=f"f1l_{hf}_{mc}")
                                nc.sync.dma_start(wh, wv["f1_hi"][:, :, ts(mc, P)])
                                nc.sync.dma_start(wl, wv["f1_lo"][:, :, ts(mc, P)])
                                ps = psum.tile([P, T2], F32, tag="mm",
                                               name=f"ps_f1_{hf}_{mc}")
                                first = True
                                for xa, wa in ((xnT_h, wh), (xnT_l, wh),
                                               (xnT_h, wl)):
                                    for kk in range(D // P):
                                        nc.tensor.matmul(
                                            ps, wa[:, kk, :], xa[:, kk, hsl],
                                            start=first,
                                            stop=(xa is xnT_h and wa is wl and
                                                  kk == D // P - 1))
                                        first = False
                                if b1_fm is not None:
                                    nc.vector.tensor_scalar(
                                        out=z1T[:, mc, hsl], in0=ps,
                                        scalar1=b1_fm[:, mc:mc + 1],
                                        scalar2=None, op0=OP.add)
                                else:
                                    nc.vector.tensor_copy(z1T[:, mc, hsl], ps)
                                zq = lp.tile([P, T2], F32R, tag="zq",
                                             name=f"zq_{hf}_{mc}")
                                nc.scalar.activation(
                                    out=zq, in_=z1T[:, mc, hsl],
                                    func=mybir.ActivationFunctionType.Square,
                                    bias=0.0, scale=1.0)
                                nc.tensor.matmul(pr_sq[:, hsl], ones_r, zq,
                                                 start=(mc == 0),
                                                 stop=(mc == NMC - 1))

                            # stats for this half
                            mrow = lp.tile([1, T2], F32, tag="mrow",
                                           name=f"mrow_{hf}")
                            nc.vector.tensor_scalar(
                                out=mrow, in0=pr_sum[:, hsl],
                                scalar1=1.0 / HID, scalar2=cfg["b1_sum"] / HID,
                                op0=OP.mult, op1=OP.add)
                            e2row = lp.tile([1, T2], F32, tag="e2row",
                                            name=f"e2row_{hf}")
                            nc.vector.tensor_scalar_mul(e2row, pr_sq[:, hsl],
                                                        1.0 / HID)
                            vrow = lp.tile([1, T2], F32, tag="vrow",
                                           name=f"vrow_{hf}")
                            nc.vector.tensor_tensor(out=vrow, in0=mrow,
                                                    in1=mrow, op=OP.mult)
                            nc.vector.tensor_tensor(out=vrow, in0=e2row,
                                                    in1=vrow, op=OP.subtract)
                            srow = lp.tile([1, T2], F32, tag="srow",
                                           name=f"srow_{hf}")
                            nc.scalar.activation(
                                out=srow, in_=vrow,
                                func=mybir.ActivationFunctionType.Sqrt,
                                bias=eps_t[0:1], scale=1.0)
                            m_b = lp.tile([P, T2], F32, tag="m_b",
                                          name=f"m_b_{hf}")
                            s_b = lp.tile([P, T2], F32, tag="s_b",
                                          name=f"s_b_{hf}")
                            nc.gpsimd.partition_broadcast(m_b, mrow)
                            nc.gpsimd.partition_broadcast(s_b, srow)
                            for mc in range(NMC):
                                thr = thrp.tile([P, T2], F32, tag="thr",
                                                name=f"th1_{hf}_{mc}")
                                nc.vector.tensor_scalar(
                                    out=thr, in0=s_b,
                                    scalar1=t1_fm[:, mc:mc + 1], scalar2=None,
                                    op0=OP.mult)
                                nc.vector.tensor_tensor(out=thr, in0=thr,
                                                        in1=m_b, op=OP.add)
                                nc.vector.tensor_tensor(
                                    out=m1T[:, mc, hsl], in0=z1T[:, mc, hsl],
                                    in1=thr, op=cmp1)

                        if debug_outputs:
                            dbg_copy(dbg["d_z1T"], z1T, fm=True)
                # sp5 closed: xnT freed

                if debug_outputs:
                    dbg_copy(dbg["d_m1T"], m1T, fm=True)

                # ---- stage 7: fc2 ----
                with tc.tile_pool(name="z2p", bufs=1) as z2pool, \
                     tc.tile_pool(name="tfc2", bufs=1) as tpool, \
                     tc.tile_pool(name="wf2", bufs=6) as wpool, \
                     tc.tile_pool(name="fc2ln", bufs=3) as lp:
                    t2_b = load_bcast("t2", D, tpool)
                    b2_b = load_bcast("b2", D, tpool) if cfg["has_b2"] else None
                    z2 = z2pool.tile([P, T // P, D], F32, tag="z2")
                    st2 = z2pool.tile([P, T // P, D // 512, 6], F32, tag="st_fc2")
                    for n in range(D // 512):
                        pss = []
                        for _pi in range(T // P):
                            pst = psum.tile([P, 512], F32, tag="mm",
                                            name=f"ps2_{n}_{_pi}")
                            pss.append(pst)
                        for kk in range(HID // P):
                            wh = wpool.tile([P, 512], BF16, tag="f2h",
                                            name=f"f2h_{n}_{kk}")
                            wl = wpool.tile([P, 512], BF16, tag="f2l",
                                            name=f"f2l_{n}_{kk}")
                            nc.sync.dma_start(wh, wv["f2_hi"][:, kk, ts(n, 512)])
                            nc.sync.dma_start(wl, wv["f2_lo"][:, kk, ts(n, 512)])
                            for tt in range(T // P):
                                nc.tensor.matmul(pss[tt], m1T[:, kk, ts(tt, P)],
                                                 wh, start=(kk == 0), stop=False)
                                nc.tensor.matmul(pss[tt], m1T[:, kk, ts(tt, P)],
                                                 wl, start=False,
                                                 stop=(kk == HID // P - 1))
                        for tt in range(T // P):
                            if b2_b is not None:
                                nc.vector.tensor_tensor(
                                    out=z2[:, tt, ts(n, 512)], in0=pss[tt],
                                    in1=b2_b[:, ts(n, 512)], op=OP.add)
                            else:
                                nc.vector.tensor_copy(z2[:, tt, ts(n, 512)],
                                                      pss[tt])
                            nc.vector.bn_stats(st2[:, tt, n],
                                               z2[:, tt, ts(n, 512)])

                    cmp2 = OP.is_ge if cfg["mode_2"] == "pos" else OP.is_le
                    for tt in range(T // P):
                        mv = lp.tile([P, 2], F32, tag="mv2", name=f"mv2_{tt}")
                        nc.vector.bn_aggr(mv, st2[:, tt])
                        std = lp.tile([P, 1], F32, tag="sd2", name=f"sd2_{tt}")
                        nc.scalar.activation(
                            out=std, in_=mv[:, 1:2],
                            func=mybir.ActivationFunctionType.Sqrt,
                            bias=eps_t, scale=1.0)
                        for n in range(D // 512):
                            thr = thrp.tile([P, 512], F32, tag="thr",
                                            name=f"th2_{tt}_{n}")
                            nc.vector.tensor_scalar(
                                out=thr, in0=t2_b[:, ts(n, 512)],
                                scalar1=std, scalar2=mv[:, 0:1],
                                op0=OP.mult, op1=OP.add)
                            m2c = lp.tile([P, 512], F32, tag="m2c",
                                          name=f"m2c_{tt}_{n}")
                            nc.vector.tensor_tensor(
                                out=m2c, in0=z2[:, tt, ts(n, 512)],
                                in1=thr, op=cmp2)
                            ot = lp.tile([P, 512], F32, tag="ot",
                                         name=f"ot_{tt}_{n}")
                            nc.vector.tensor_tensor(
                                out=ot, in0=xnew[:, tt, ts(n, 512)],
                                in1=m2c, op=OP.add)
                            nc.sync.dma_start(
                                out_dram.ap().rearrange(
                                    "(c p) f -> p c f", p=P)[:, tt, ts(n, 512)],
                                ot)

    nc.compile()
    return nc


def _sign_mode(g):
    if np.all(g > 0):
        return "pos"
    if np.all(g < 0):
        return "neg"
    raise NotImplementedError("mixed-sign LN gain not supported")


def make_core_inputs(x, q_w, q_g, q_b, k_w, k_g, k_b, v_w, v_g, v_b,
                     proj_w, proj_bias, proj_g, proj_beta,
                     fc1_w, fc1_bias, fc1_g, fc1_beta,
                     fc2_w, fc2_bias, fc2_g, fc2_beta):
    f32 = np.float32
    X = np.asarray(x, f32).reshape(B * L, D)

    wsplit = {}
    for nm, W in (("qw", q_w), ("kw", k_w), ("vw", v_w),
                  ("pw", proj_w), ("f1", fc1_w), ("f2", fc2_w)):
        hi, lo = _split_hi_lo(np.asarray(W, f32))
        wsplit[f"{nm}_hi"] = hi
        wsplit[f"{nm}_lo"] = lo

    def thrvec(g, b):
        return ((THETA - np.asarray(b, np.float64))
                / np.asarray(g, np.float64)).astype(f32)

    thr = {"tq": thrvec(q_g, q_b), "tk": thrvec(k_g, k_b),
           "tv": thrvec(v_g, v_b), "tp": thrvec(proj_g, proj_beta),
           "t1": thrvec(fc1_g, fc1_beta), "t2": thrvec(fc2_g, fc2_beta)}

    ws1 = np.asarray(fc1_w, np.float64).sum(axis=1).astype(f32)
    ws1_hi, ws1_lo = _split_hi_lo(ws1)
    cfg = {
        "b1_sum": float(np.asarray(fc1_bias, np.float64).sum()),
        "mode_q": _sign_mode(np.asarray(q_g)), "mode_k": _sign_mode(np.asarray(k_g)),
        "mode_v": _sign_mode(np.asarray(v_g)), "mode_p": _sign_mode(np.asarray(proj_g)),
        "mode_1": _sign_mode(np.asarray(fc1_g)), "mode_2": _sign_mode(np.asarray(fc2_g)),
        "has_bp": bool(np.any(np.asarray(proj_bias) != 0)),
        "has_b1": bool(np.any(np.asarray(fc1_bias) != 0)),
        "has_b2": bool(np.any(np.asarray(fc2_bias) != 0)),
    }
    biases = {"bp": np.asarray(proj_bias, f32), "b1": np.asarray(fc1_bias, f32),
              "b2": np.asarray(fc2_bias, f32)}

    in_maps = []
    for c in range(NCORES):
        b = c // 2
        h = c % 2
        own = X[b * L + h * T: b * L + (h + 1) * T]
        other = X[b * L + (1 - h) * T: b * L + (2 - h) * T]
        Xp = np.concatenate([own, other], axis=0)           # [TB, D] own-first
        xT = np.ascontiguousarray(Xp.T)                     # [D, TB]
        xT_hi, xT_lo = _split_hi_lo(xT)
        m = {"xT_hi": xT_hi, "xT_lo": xT_lo,
             "x_tok": np.ascontiguousarray(own),
             "ident": np.eye(P, dtype=np.float32).astype(ml_dtypes.bfloat16),
             "ws1_hi": ws1_hi, "ws1_lo": ws1_lo}
        m.update(wsplit)
        m.update(thr)
        for nm in ("bp", "b1", "b2"):
            if cfg[f"has_{nm}"]:
                m[nm] = biases[nm]
        in_maps.append(m)
    return in_maps, cfg


_prog_cache = {}


def kernel(**inputs) -> np.ndarray:
    in_maps, cfg = make_core_inputs(**inputs)
    key = tuple(sorted(cfg.items()))
    if key not in _prog_cache:
        _prog_cache[key] = build_program(cfg)
    nc = _prog_cache[key]

    res = run_bass_kernel_spmd(nc, in_maps, core_ids=list(range(NCORES)))
    last_run_info["exec_time_ns"] = res.exec_time_ns
    last_run_info["mean_exec_time_ns"] = res.mean_exec_time_ns

    out = np.empty((B, L, D), np.float32)
    for c in range(NCORES):
        b = c // 2
        h = c % 2
        out[b, h * T:(h + 1) * T, :] = res.results[c]["out"]
    return out


# revision 38
# speedup vs baseline: 1.0331x; 1.0331x over previous
"""Trainium2 Bass kernel for nn_Block_80041010528755 (spiking transformer block).

Math structure (see reference):
  q = spike(LN(x@q_w) >= 2), k/v likewise (binary {0,1})
  attn has NO softmax -> (q@k^T)@v == q@(k^T@v): per-head 64x64 kv matrix,
  exact in bf16/fp32 because spikes are binary and sums are small integers.
  y2 = spike(LN(yspike@proj_w + pb) >= 2); x' = x + y2
  m1 = spike(LN(x'@fc1_w + b1) >= 2); m2 = spike(LN(m1@fc2_w + b2) >= 2)
  out = x' + m2

Precision: fp32-input matmuls (q/k/v from x, fc1 from x') use 3-product
bf16 hi/lo splits (x_hi@W_hi + x_lo@W_hi + x_hi@W_lo, ~2^-16 rel);
binary-input matmuls (proj, fc2) use 2 products (S@W_hi + S@W_lo).
All accumulate in fp32 PSUM.

Sharding: 8-way token-parallel, 512 tokens/core (half a batch). k/v are
computed over the core's full 1024-token batch (duplicated within the
core pair) so attention needs no collectives.
"""

import os
import sys

for _p in ("/root/.axon_site/_ro/trn_rl_repo", "/opt/trn_rl_repo"):
    if os.path.isdir(_p) and _p not in sys.path:
        sys.path.append(_p)

import numpy as np
import ml_dtypes

import concourse.bass as bass
import concourse.bacc as bacc
import concourse.tile as tile
import concourse.mybir as mybir
from concourse.bass import ts
from concourse.bass_utils import run_bass_kernel_spmd

F32 = mybir.dt.float32
BF16 = mybir.dt.bfloat16
OP = mybir.AluOpType

B, L, D = 4, 1024, 1024
HID = 4096
H, HD = 16, 64
NCORES = 8
T = 512          # own tokens per core
TB = 1024        # batch tokens per core (own + partner half)
P = 128
LN_EPS = 1e-5
THETA = 2.0      # LN-spike threshold: TAU*v_th = 2*1
ATTN_THETA = 1.0  # attn spike: y >= TAU*0.5

# module-global stash for timing info from the last kernel() call
last_run_info = {}


def _split_hi_lo(a32):
    hi = a32.astype(ml_dtypes.bfloat16)
    lo = (a32 - hi.astype(np.float32)).astype(ml_dtypes.bfloat16)
    return np.ascontiguousarray(hi), np.ascontiguousarray(lo)


def _bcast_ap(dram_ap, parts=P):
    """[D] dram tensor viewed as [parts, D] with 0-stride partitions."""
    return bass.AP(tensor=dram_ap.tensor, offset=dram_ap.offset,
                   ap=[[0, parts]] + list(dram_ap.ap))


def build_program(cfg, debug_outputs=False):
    """cfg: dict with has_bias flags + g-sign modes per LN stage."""
    nc = bacc.Bacc("TRN2", target_bir_lowering=False, debug=False)

    # ---- DRAM tensors ----
    xT_hi = nc.dram_tensor("xT_hi", [D, TB], BF16, kind="ExternalInput")
    xT_lo = nc.dram_tensor("xT_lo", [D, TB], BF16, kind="ExternalInput")
    x_tok = nc.dram_tensor("x_tok", [T, D], F32, kind="ExternalInput")

    w_names = {}
    for nm, (din, dout) in (("qw", (D, D)), ("kw", (D, D)), ("vw", (D, D)),
                            ("pw", (D, D)), ("f1", (D, HID)), ("f2", (HID, D))):
        for h in ("hi", "lo"):
            w_names[f"{nm}_{h}"] = nc.dram_tensor(
                f"{nm}_{h}", [din, dout], BF16, kind="ExternalInput")

    thr_names = {}
    for nm, dd in (("tq", D), ("tk", D), ("tv", D), ("tp", D),
                   ("t1", HID), ("t2", D)):
        thr_names[nm] = nc.dram_tensor(nm, [dd], F32, kind="ExternalInput")

    ident_in = nc.dram_tensor("ident", [P, P], BF16, kind="ExternalInput")
    ws1_hi = nc.dram_tensor("ws1_hi", [D], BF16, kind="ExternalInput")
    ws1_lo = nc.dram_tensor("ws1_lo", [D], BF16, kind="ExternalInput")

    bias_names = {}
    for nm, dd in (("bp", D), ("b1", HID), ("b2", D)):
        if cfg[f"has_{nm}"]:
            bias_names[nm] = nc.dram_tensor(nm, [dd], F32, kind="ExternalInput")

    out_dram = nc.dram_tensor("out", [T, D], F32, kind="ExternalOutput")

    dbg = {}
    if debug_outputs:
        for nm, shp, dt in (("d_qsT", [D, T], BF16), ("d_ks", [TB, D], BF16),
                            ("d_vs", [TB, D], BF16), ("d_ysT", [D, T], BF16),
                            ("d_y2", [T, D], BF16), ("d_m1T", [HID, T], BF16),
                            ("d_z1T", [HID, T], F32)):
            dbg[nm] = nc.dram_tensor(nm, shp, dt, kind="ExternalOutput")

    # weight dram views [p, kc, dout]
    wv = {k: v.ap().rearrange("(kc p) f -> p kc f", p=P)
          for k, v in w_names.items()}

    def dbg_copy(dram, sb, fm=False):
        pat = "(c p) t -> p c t" if fm else "(c p) f -> p c f"
        dv = dram.ap().rearrange(pat, p=P)
        for c in range(sb.shape[1]):
            nc.sync.dma_start(dv[:, c, :], sb[:, c, :])

    with tile.TileContext(nc) as tc:
        with tc.tile_pool(name="psum", bufs=6, space="PSUM") as psum, \
             tc.tile_pool(name="stats", bufs=6) as stats, \
             tc.tile_pool(name="thrp", bufs=3) as thrp, \
             tc.tile_pool(name="consts", bufs=1) as consts, \
             tc.tile_pool(name="resid", bufs=1) as resid:

            eps_t = consts.tile([P, 1], F32)
            nc.vector.memset(eps_t, LN_EPS)
            ident = consts.tile([P, P], BF16, tag="ident")
            nc.sync.dma_start(ident, ident_in.ap())
            ones0 = consts.tile([P, 1], F32, tag="ones0")
            nc.vector.memset(ones0, 1.0)
            ones_r = consts.tile([P, 1], mybir.dt.float32r, tag="ones_r")
            nc.vector.tensor_copy(ones_r, ones0)

            # round-robin 128x128 DMA transposes over the two HWDGE queues
            _tp_state = [0]

            def tpose(dst, src):
                eng = (nc.sync, nc.sync)[_tp_state[0] % 2]
                _tp_state[0] += 1
                eng.dma_start(out=dst, in_=src, transpose=True)

            def load_bcast(name, dd, pool):
                t = pool.tile([P, dd], F32, tag=f"bc_{name}", name=f"bc_{name}")
                nc.gpsimd.dma_start(t, _bcast_ap(thr_names[name].ap()
                                                 if name in thr_names
                                                 else bias_names[name].ap()))
                return t

            # ---------- LN + spike helper ----------
            def ln_spike(z_chunks, thr_b, mode, out_fn, stat_tag):
                """z_chunks: APs [128, 512] covering the feature dim for one
                token block. thr_b: [128, d_total] bcast of (theta-b)/g.
                Emits spike = cmp(z, mean + t*std) per chunk via out_fn."""
                nchunks = len(z_chunks)
                st = stats.tile([P, nchunks, 6], F32, tag=f"st_{stat_tag}",
                                name=f"st_{stat_tag}")
                for j, zc in enumerate(z_chunks):
                    nc.vector.bn_stats(st[:, j], zc)
                mv = stats.tile([P, 2], F32, tag=f"mv_{stat_tag}",
                                name=f"mv_{stat_tag}")
                nc.vector.bn_aggr(mv, st)
                std = stats.tile([P, 1], F32, tag=f"sd_{stat_tag}",
                                 name=f"sd_{stat_tag}")
                nc.scalar.activation(out=std, in_=mv[:, 1:2],
                                     func=mybir.ActivationFunctionType.Sqrt,
                                     bias=eps_t, scale=1.0)
                cmp = OP.is_ge if mode == "pos" else OP.is_le
                for j, zc in enumerate(z_chunks):
                    thr = thrp.tile([P, 512], F32, tag="thr", name=f"th_{stat_tag}_{j}")
                    nc.vector.tensor_scalar(out=thr, in0=thr_b[:, ts(j, 512)],
                                            scalar1=std, scalar2=mv[:, 0:1],
                                            op0=OP.mult, op1=OP.add)
                    out_fn(j, zc, thr, cmp)

            xnew = resid.tile([P, T // P, D], F32, tag="xnew")

            with tc.tile_pool(name="xtp", bufs=1) as xtp:
                xt = xtp.tile([P, T // P, D], F32, tag="xt")
                # off the sync queue: only needed at proj time
                nc.gpsimd.dma_start(xt, x_tok.ap().rearrange("(c p) f -> p c f",
                                                             p=P))

                with tc.tile_pool(name="sp3", bufs=1) as sp3:
                    ysT = sp3.tile([P, D // P, T], BF16, tag="ysT")
                    y2 = sp3.tile([P, T // P, D], BF16, tag="y2")

                    with tc.tile_pool(name="sp12", bufs=1) as sp12:
                        kS = sp12.tile([P, TB // P, D], BF16, tag="kS")
                        vS = sp12.tile([P, TB // P, D], BF16, tag="vS")
                        qTS = sp12.tile([P, D // P, T], BF16, tag="qTS")

                        # ======== stage 1+2: q (own), k, v (batch) =========
                        with tc.tile_pool(name="xTp", bufs=1) as xTpool, \
                             tc.tile_pool(name="tqkv", bufs=1) as tpool, \
                             tc.tile_pool(name="qsc", bufs=3) as qscp, \
                             tc.tile_pool(name="wqkvh", bufs=2) as wqkvh, \
                             tc.tile_pool(name="wqkvl", bufs=1) as wqkvl:
                            xTh = xTpool.tile([P, D // P, TB], BF16, tag="xTh")
                            xTl = xTpool.tile([P, D // P, TB], BF16, tag="xTl")
                            xThd = xT_hi.ap().rearrange("(c p) t -> p c t", p=P)
                            xTld = xT_lo.ap().rearrange("(c p) t -> p c t", p=P)
                            # own-token halves first (q only needs these),
                            # partner halves after the q weights
                            nc.sync.dma_start(xTh[:, :, 0:T], xThd[:, :, 0:T])
                            nc.sync.dma_start(xTl[:, :, 0:T], xTld[:, :, 0:T])
                            # bcast thresholds off the sync queue
                            tq_b = load_bcast("tq", D, tpool)
                            tk_b = load_bcast("tk", D, tpool)
                            tv_b = load_bcast("tv", D, tpool)

                            for nm, spk, thr_b, ntt, mode in (
                                    ("qw", None, tq_b, T // P, cfg["mode_q"]),
                                    ("kw", kS, tk_b, TB // P, cfg["mode_k"]),
                                    ("vw", vS, tv_b, TB // P, cfg["mode_v"])):
                                if True:
                                    whi = wqkvh.tile([P, D // P, D], BF16,
                                                     tag="wqkv_hi",
                                                     name=f"{nm}_hi_t")
                                    wlo = wqkvl.tile([P, D // P, D], BF16,
                                                     tag="wqkv_lo",
                                                     name=f"{nm}_lo_t")
                                    nc.sync.dma_start(whi, wv[f"{nm}_hi"])
                                    nc.sync.dma_start(wlo, wv[f"{nm}_lo"])
                                    if nm == "qw":
                                        # partner xT halves: needed from k on
                                        nc.sync.dma_start(xTh[:, :, T:TB],
                                                          xThd[:, :, T:TB])
                                        nc.sync.dma_start(xTl[:, :, T:TB],
                                                          xTld[:, :, T:TB])
                                    for tt in range(ntt):
                                        pss = []
                                        for n in range(D // 512):
                                            ps = psum.tile([P, 512], F32, tag="mm",
                                                           name=f"ps_{nm}_{tt}_{n}")
                                            first = True
                                            for xa, wa in ((xTh, whi), (xTl, whi),
                                                           (xTh, wlo)):
                                                for kk in range(D // P):
                                                    nc.tensor.matmul(
                                                        ps, xa[:, kk, ts(tt, P)],
                                                        wa[:, kk, ts(n, 512)],
                                                        start=first,
                                                        stop=(xa is xTh and
                                                              wa is wlo and
                                                              kk == D // P - 1))
                                                    first = False
                                            pss.append(ps)

                                        if spk is None:
                                            # q: emit to scratch, PE-transpose
                                            # to feature-major immediately
                                            def emit(j, zc, thc, cmp, tt=tt):
                                                qc = qscp.tile(
                                                    [P, 512], BF16, tag="qc",
                                                    name=f"qc_{tt}_{j}")
                                                nc.vector.tensor_tensor(
                                                    out=qc, in0=zc, in1=thc,
                                                    op=cmp)
                                                for j2 in range(4):
                                                    fcx = j * 4 + j2
                                                    pt = psum.tile(
                                                        [P, P], BF16, tag="mm",
                                                        name=f"qpt_{tt}_{fcx}")
                                                    nc.tensor.transpose(
                                                        pt, qc[:, ts(j2, P)],
                                                        ident)
                                                    nc.vector.tensor_copy(
                                                        qTS[:, fcx, ts(tt, P)],
                                                        pt)
                                        else:
                                            def emit(j, zc, thc, cmp, spk=spk,
                                                     tt=tt):
                                                nc.vector.tensor_tensor(
                                                    out=spk[:, tt, ts(j, 512)],
                                                    in0=zc, in1=thc, op=cmp)
                                        ln_spike(pss, thr_b, mode, emit, "qkv")

                        if debug_outputs:
                            dbg_copy(dbg["d_qsT"], qTS, fm=True)
                            dbg_copy(dbg["d_ks"], kS)
                            dbg_copy(dbg["d_vs"], vS)

                        # ======== stage 3: kv + y + attn spike =============
                        with tc.tile_pool(name="attn", bufs=4) as apool:
                            for hp in range(D // P):   # 8 head pairs
                                pkv = psum.tile([P, P], F32, tag="mm",
                                                name=f"pkv_{hp}")
                                for tt in range(TB // P):
                                    nc.tensor.matmul(
                                        pkv, kS[:, tt, ts(hp, P)],
                                        vS[:, tt, ts(hp, P)],
                                        start=(tt == 0), stop=(tt == TB // P - 1))
                                kvd = apool.tile([P, P], F32, tag="kvd",
                                                 name=f"kvd_{hp}")
                                nc.vector.memset(kvd, 0.0)
                                nc.vector.tensor_scalar_mul(
                                    kvd[0:HD, 0:HD], pkv[0:HD, 0:HD], 0.125)
                                nc.vector.tensor_scalar_mul(
                                    kvd[HD:P, HD:P], pkv[HD:P, HD:P], 0.125)
                                kvh = apool.tile([P, P], BF16, tag="kvh",
                                                 name=f"kvh_{hp}")
                                nc.vector.tensor_copy(kvh, kvd)
                                kvhf = apool.tile([P, P], F32, tag="kvhf",
                                                  name=f"kvhf_{hp}")
                                nc.vector.tensor_copy(kvhf, kvh)
                                kvl = apool.tile([P, P], BF16, tag="kvl",
                                                 name=f"kvl_{hp}")
                                nc.vector.tensor_tensor(out=kvl, in0=kvd,
                                                        in1=kvhf, op=OP.subtract)
                                py = psum.tile([P, T], F32, tag="mm",
                                               name=f"py_{hp}")
                                nc.tensor.matmul(py, kvh, qTS[:, hp, :],
                                                 start=True, stop=False)
                                nc.tensor.matmul(py, kvl, qTS[:, hp, :],
                                                 start=False, stop=True)
                                nc.vector.tensor_scalar(out=ysT[:, hp, :],
                                                        in0=py,
                                                        scalar1=ATTN_THETA,
                                                        scalar2=None,
                                                        op0=OP.is_ge)
                    # sp12 closed: qS/kS/vS/qTS freed

                    if debug_outputs:
                        dbg_copy(dbg["d_ysT"], ysT, fm=True)

                    # ======== stage 4: proj + LN + spike, residual =========
                    with tc.tile_pool(name="wp", bufs=1) as wpool, \
                         tc.tile_pool(name="tproj", bufs=1) as tpool, \
                         tc.tile_pool(name="zproj", bufs=4) as zpool:
                        tp_b = load_bcast("tp", D, tpool)
                        bp_b = load_bcast("bp", D, tpool) if cfg["has_bp"] else None
                        pwh = wpool.tile([P, D // P, D], BF16, tag="w_pw_hi")
                        pwl = wpool.tile([P, D // P, D], BF16, tag="w_pw_lo")
                        nc.sync.dma_start(pwh, wv["pw_hi"])
                        nc.sync.dma_start(pwl, wv["pw_lo"])
                        for tt in range(T // P):
                            zrefs = []
                            for n in range(D // 512):
                                ps = psum.tile([P, 512], F32, tag="mm",
                                               name=f"ps_pr_{tt}_{n}")
                                first = True
                                for wa in (pwh, pwl):
                                    for kk in range(D // P):
                                        nc.tensor.matmul(
                                            ps, ysT[:, kk, ts(tt, P)],
                                            wa[:, kk, ts(n, 512)],
                                            start=first,
                                            stop=(wa is pwl and kk == D // P - 1))
                                        first = False
                                if bp_b is not None:
                                    zc = zpool.tile([P, 512], F32, tag="zproj",
                                                    name=f"zpr_{tt}_{n}")
                                    nc.vector.tensor_tensor(
                                        out=zc, in0=ps,
                                        in1=bp_b[:, ts(n, 512)], op=OP.add)
                                    zrefs.append(zc)
                                else:
                                    zrefs.append(ps)

                            def emit(j, zc, thc, cmp, tt=tt):
                                nc.vector.tensor_tensor(
                                    out=y2[:, tt, ts(j, 512)],
                                    in0=zc, in1=thc, op=cmp)
                            ln_spike(zrefs, tp_b, cfg["mode_p"], emit, "proj")
                            nc.vector.tensor_tensor(out=xnew[:, tt, :],
                                                    in0=xt[:, tt, :],
                                                    in1=y2[:, tt, :], op=OP.add)

                    if debug_outputs:
                        dbg_copy(dbg["d_y2"], y2)
                # sp3 closed: ysT, y2 freed
            # xtp closed: xt freed

            # ============ stage 5+6+7 ======================================
            with tc.tile_pool(name="sp6", bufs=1) as sp6:
                m1T = sp6.tile([P, HID // P, T], BF16, tag="m1T")

                with tc.tile_pool(name="sp5", bufs=1) as sp5:
                    xnT_h = sp5.tile([P, D // P, T], BF16, tag="xnT_h")
                    xnT_l = sp5.tile([P, D // P, T], BF16, tag="xnT_l")
                    # ---- stage 5: split xnew + PE transpose ----
                    with tc.tile_pool(name="xsplit", bufs=3) as xsp:
                        for tt in range(T // P):
                            xh = xsp.tile([P, D], BF16, tag="xh", name=f"xh_{tt}")
                            xl = xsp.tile([P, D], BF16, tag="xl", name=f"xl_{tt}")
                            xhf = xsp.tile([P, D], F32, tag="xhf", name=f"xhf_{tt}")
                            nc.vector.tensor_copy(xh, xnew[:, tt, :])
                            nc.vector.tensor_copy(xhf, xh)
                            nc.vector.tensor_tensor(out=xl, in0=xnew[:, tt, :],
                                                    in1=xhf, op=OP.subtract)
                            for fc in range(D // P):
                                for src, dst in ((xh, xnT_h), (xl, xnT_l)):
                                    pt = psum.tile([P, P], BF16, tag="mm",
                                                   name=f"pt_{tt}_{fc}")
                                    nc.tensor.transpose(pt, src[:, ts(fc, P)],
                                                        ident)
                                    nc.vector.tensor_copy(
                                        dst[:, fc, ts(tt, P)], pt)

                    # ---- stage 6: fc1, FEATURE-major so m1 spikes land
                    #      directly in fc2's lhsT layout (no transposes).
                    #      LN stats via fp32r ones-matmul reductions. ----
                    F32R = mybir.dt.float32r
                    NMC = HID // P   # 32 dout chunks
                    with tc.tile_pool(name="z1p", bufs=1) as z1pool, \
                         tc.tile_pool(name="tfc1", bufs=1) as tpool, \
                         tc.tile_pool(name="wf1", bufs=2) as wpool, \
                         tc.tile_pool(name="psred", bufs=1, space="PSUM") as psr, \
                         tc.tile_pool(name="fc1ln", bufs=4) as lp:
                        # t1 / b1 as per-partition [128, 32] (feature-major)
                        t1_fm = tpool.tile([P, NMC], F32, tag="t1_fm")
                        nc.sync.dma_start(
                            t1_fm, thr_names["t1"].ap().rearrange(
                                "(c p) -> p c", p=P))
                        b1_fm = None
                        if cfg["has_b1"]:
                            b1_fm = tpool.tile([P, NMC], F32, tag="b1_fm")
                            nc.sync.dma_start(
                                b1_fm, bias_names["b1"].ap().rearrange(
                                    "(c p) -> p c", p=P))
                        z1T = z1pool.tile([P, NMC, T], F32, tag="z1T")
                        pr_sum = psr.tile([1, T], F32, tag="pr_sum")
                        pr_sq = psr.tile([1, T], F32, tag="pr_sq")
                        cmp1 = OP.is_ge if cfg["mode_1"] == "pos" else OP.is_le

                        # mean*HID = xn @ rowsum(fc1_w) (+ sum(b1), host-folded)
                        wsh = tpool.tile([P, D // P], BF16, tag="ws1h")
                        wsl = tpool.tile([P, D // P], BF16, tag="ws1l")
                        nc.sync.dma_start(wsh, ws1_hi.ap().rearrange(
                            "(c p) -> p c", p=P))
                        nc.sync.dma_start(wsl, ws1_lo.ap().rearrange(
                            "(c p) -> p c", p=P))
                        # two token halves: half 0's LN-apply overlaps
                        # half 1's matmuls (fc1 weights streamed twice)
                        T2 = T // 2
                        for hf in range(2):
                            hsl = bass.ds(hf * T2, T2)
                            first = True
                            for xa, wa in ((xnT_h, wsh), (xnT_l, wsh),
                                           (xnT_h, wsl)):
                                for kk in range(D // P):
                                    nc.tensor.matmul(
                                        pr_sum[:, hsl], wa[:, kk:kk + 1],
                                        xa[:, kk, hsl],
                                        start=first,
                                        stop=(xa is xnT_h and wa is wsl and
                                              kk == D // P - 1))
                                    first = False

                            for mc in range(NMC):
                                if mc % 4 == 0:
                                    # batched weight load: 4 dout chunks
                                    w4h = wpool.tile([P, D // P, 4 * P], BF16,
                                                     tag="f1h",
                                                     name=f"f1h_{hf}_{mc}")
                                    w4l = wpool.tile([P, D // P, 4 * P], BF16,
                                                     tag="f1l",
                                                     name=f"f1l_{hf}_{mc}")
                                    nc.sync.dma_start(
                                        w4h, wv["f1_hi"][:, :, ts(mc // 4, 4 * P)])
                                    nc.sync.dma_start(
                                        w4l, wv["f1_lo"][:, :, ts(mc // 4, 4 * P)])
                                wh = w4h[:, :, ts(mc % 4, P)]
                                wl = w4l[:, :, ts(mc % 4, P)]
                                ps = psum.tile([P, T2], F32, tag="mm",
                                               name=f"ps_f1_{hf}_{mc}")
                                first = True
                                for xa, wa in ((xnT_h, wh), (xnT_l, wh),
                                               (xnT_h, wl)):
                                    for kk in range(D // P):
                                        nc.tensor.matmul(
                                            ps, wa[:, kk, :], xa[:, kk, hsl],
                                            start=first,
                                            stop=(xa is xnT_h and wa is wl and
                                                  kk == D // P - 1))
                                        first = False
                                if b1_fm is not None:
                                    nc.vector.tensor_scalar(
                                        out=z1T[:, mc, hsl], in0=ps,
                                        scalar1=b1_fm[:, mc:mc + 1],
                                        scalar2=None, op0=OP.add)
                                else:
                                    nc.vector.tensor_copy(z1T[:, mc, hsl], ps)
                                zq = lp.tile([P, T2], F32R, tag="zq",
                                             name=f"zq_{hf}_{mc}")
                                nc.scalar.activation(
                                    out=zq, in_=z1T[:, mc, hsl],
                                    func=mybir.ActivationFunctionType.Square,
                                    bias=0.0, scale=1.0)
                                nc.tensor.matmul(pr_sq[:, hsl], ones_r, zq,
                                                 start=(mc == 0),
                                                 stop=(mc == NMC - 1))

                            # stats for this half
                            mrow = lp.tile([1, T2], F32, tag="mrow",
                                           name=f"mrow_{hf}")
                            nc.vector.tensor_scalar(
                                out=mrow, in0=pr_sum[:, hsl],
                                scalar1=1.0 / HID, scalar2=cfg["b1_sum"] / HID,
                                op0=OP.mult, op1=OP.add)
                            e2row = lp.tile([1, T2], F32, tag="e2row",
                                            name=f"e2row_{hf}")
                            nc.vector.tensor_scalar_mul(e2row, pr_sq[:, hsl],
                                                        1.0 / HID)
                            vrow = lp.tile([1, T2], F32, tag="vrow",
                                           name=f"vrow_{hf}")
                            nc.vector.tensor_tensor(out=vrow, in0=mrow,
                                                    in1=mrow, op=OP.mult)
                            nc.vector.tensor_tensor(out=vrow, in0=e2row,
                                                    in1=vrow, op=OP.subtract)
                            srow = lp.tile([1, T2], F32, tag="srow",
                                           name=f"srow_{hf}")
                            nc.scalar.activation(
                                out=srow, in_=vrow,
                                func=mybir.ActivationFunctionType.Sqrt,
                                bias=eps_t[0:1], scale=1.0)
                            m_b = lp.tile([P, T2], F32, tag="m_b",
                                          name=f"m_b_{hf}")
                            s_b = lp.tile([P, T2], F32, tag="s_b",
                                          name=f"s_b_{hf}")
                            nc.gpsimd.partition_broadcast(m_b, mrow)
                            nc.gpsimd.partition_broadcast(s_b, srow)
                            for mc in range(NMC):
                                thr = thrp.tile([P, T2], F32, tag="thr",
                                                name=f"th1_{hf}_{mc}")
                                nc.vector.tensor_scalar(
                                    out=thr, in0=s_b,
                                    scalar1=t1_fm[:, mc:mc + 1], scalar2=None,
                                    op0=OP.mult)
                                nc.vector.tensor_tensor(out=thr, in0=thr,
                                                        in1=m_b, op=OP.add)
                                nc.vector.tensor_tensor(
                                    out=m1T[:, mc, hsl], in0=z1T[:, mc, hsl],
                                    in1=thr, op=cmp1)

                        if debug_outputs:
                            dbg_copy(dbg["d_z1T"], z1T, fm=True)
                # sp5 closed: xnT freed

                if debug_outputs:
                    dbg_copy(dbg["d_m1T"], m1T, fm=True)

                # ---- stage 7: fc2 ----
                with tc.tile_pool(name="z2p", bufs=1) as z2pool, \
                     tc.tile_pool(name="tfc2", bufs=1) as tpool, \
                     tc.tile_pool(name="wf2", bufs=3) as wpool, \
                     tc.tile_pool(name="fc2ln", bufs=3) as lp:
                    t2_b = load_bcast("t2", D, tpool)
                    b2_b = load_bcast("b2", D, tpool) if cfg["has_b2"] else None
                    z2 = z2pool.tile([P, T // P, D], F32, tag="z2")
                    st2 = z2pool.tile([P, T // P, D // 512, 6], F32, tag="st_fc2")
                    for n in range(D // 512):
                        pss = []
                        for _pi in range(T // P):
                            pst = psum.tile([P, 512], F32, tag="mm",
                                            name=f"ps2_{n}_{_pi}")
                            pss.append(pst)
                        for kk in range(HID // P):
                            if kk % 4 == 0:
                                w4h = wpool.tile([P, 4, 512], BF16, tag="f2h",
                                                 name=f"f2h_{n}_{kk}")
                                w4l = wpool.tile([P, 4, 512], BF16, tag="f2l",
                                                 name=f"f2l_{n}_{kk}")
                                nc.sync.dma_start(
                                    w4h, wv["f2_hi"][:, bass.ds(kk, 4),
                                                     ts(n, 512)])
                                nc.sync.dma_start(
                                    w4l, wv["f2_lo"][:, bass.ds(kk, 4),
                                                     ts(n, 512)])
                            wh = w4h[:, kk % 4]
                            wl = w4l[:, kk % 4]
                            for tt in range(T // P):
                                nc.tensor.matmul(pss[tt], m1T[:, kk, ts(tt, P)],
                                                 wh, start=(kk == 0), stop=False)
                                nc.tensor.matmul(pss[tt], m1T[:, kk, ts(tt, P)],
                                                 wl, start=False,
                                                 stop=(kk == HID // P - 1))
                        for tt in range(T // P):
                            if b2_b is not None:
                                nc.vector.tensor_tensor(
                                    out=z2[:, tt, ts(n, 512)], in0=pss[tt],
                                    in1=b2_b[:, ts(n, 512)], op=OP.add)
                            else:
                                nc.vector.tensor_copy(z2[:, tt, ts(n, 512)],
                                                      pss[tt])
                            nc.vector.bn_stats(st2[:, tt, n],
                                               z2[:, tt, ts(n, 512)])

                    cmp2 = OP.is_ge if cfg["mode_2"] == "pos" else OP.is_le
                    for tt in range(T // P):
                        mv = lp.tile([P, 2], F32, tag="mv2", name=f"mv2_{tt}")
                        nc.vector.bn_aggr(mv, st2[:, tt])
                        std = lp.tile([P, 1], F32, tag="sd2", name=f"sd2_{tt}")
                        nc.scalar.activation(
                            out=std, in_=mv[:, 1:2],
                            func=mybir.ActivationFunctionType.Sqrt,
                            bias=eps_t, scale=1.0)
                        for n in range(D // 512):
                            thr = thrp.tile([P, 512], F32, tag="thr",
                                            name=f"th2_{tt}_{n}")
                            nc.vector.tensor_scalar(
                                out=thr, in0=t2_b[:, ts(n, 512)],
                                scalar1=std, scalar2=mv[:, 0:1],
                                op0=OP.mult, op1=OP.add)
                            m2c = lp.tile([P, 512], F32, tag="m2c",
                                          name=f"m2c_{tt}_{n}")
                            nc.vector.tensor_tensor(
                                out=m2c, in0=z2[:, tt, ts(n, 512)],
                                in1=thr, op=cmp2)
                            ot = lp.tile([P, 512], F32, tag="ot",
                                         name=f"ot_{tt}_{n}")
                            nc.vector.tensor_tensor(
                                out=ot, in0=xnew[:, tt, ts(n, 512)],
                                in1=m2c, op=OP.add)
                            nc.sync.dma_start(
                                out_dram.ap().rearrange(
                                    "(c p) f -> p c f", p=P)[:, tt, ts(n, 512)],
                                ot)

    nc.compile()
    return nc


def _sign_mode(g):
    if np.all(g > 0):
        return "pos"
    if np.all(g < 0):
        return "neg"
    raise NotImplementedError("mixed-sign LN gain not supported")


def make_core_inputs(x, q_w, q_g, q_b, k_w, k_g, k_b, v_w, v_g, v_b,
                     proj_w, proj_bias, proj_g, proj_beta,
                     fc1_w, fc1_bias, fc1_g, fc1_beta,
                     fc2_w, fc2_bias, fc2_g, fc2_beta):
    f32 = np.float32
    X = np.asarray(x, f32).reshape(B * L, D)

    wsplit = {}
    for nm, W in (("qw", q_w), ("kw", k_w), ("vw", v_w),
                  ("pw", proj_w), ("f1", fc1_w), ("f2", fc2_w)):
        hi, lo = _split_hi_lo(np.asarray(W, f32))
        wsplit[f"{nm}_hi"] = hi
        wsplit[f"{nm}_lo"] = lo

    def thrvec(g, b):
        return ((THETA - np.asarray(b, np.float64))
                / np.asarray(g, np.float64)).astype(f32)

    thr = {"tq": thrvec(q_g, q_b), "tk": thrvec(k_g, k_b),
           "tv": thrvec(v_g, v_b), "tp": thrvec(proj_g, proj_beta),
           "t1": thrvec(fc1_g, fc1_beta), "t2": thrvec(fc2_g, fc2_beta)}

    ws1 = np.asarray(fc1_w, np.float64).sum(axis=1).astype(f32)
    ws1_hi, ws1_lo = _split_hi_lo(ws1)
    cfg = {
        "b1_sum": float(np.asarray(fc1_bias, np.float64).sum()),
        "mode_q": _sign_mode(np.asarray(q_g)), "mode_k": _sign_mode(np.asarray(k_g)),
        "mode_v": _sign_mode(np.asarray(v_g)), "mode_p": _sign_mode(np.asarray(proj_g)),
        "mode_1": _sign_mode(np.asarray(fc1_g)), "mode_2": _sign_mode(np.asarray(fc2_g)),
        "has_bp": bool(np.any(np.asarray(proj_bias) != 0)),
        "has_b1": bool(np.any(np.asarray(fc1_bias) != 0)),
        "has_b2": bool(np.any(np.asarray(fc2_bias) != 0)),
    }
    biases = {"bp": np.asarray(proj_bias, f32), "b1": np.asarray(fc1_bias, f32),
              "b2": np.asarray(fc2_bias, f32)}

    in_maps = []
    for c in range(NCORES):
        b = c // 2
        h = c % 2
        own = X[b * L + h * T: b * L + (h + 1) * T]
        other = X[b * L + (1 - h) * T: b * L + (2 - h) * T]
        Xp = np.concatenate([own, other], axis=0)           # [TB, D] own-first
        xT = np.ascontiguousarray(Xp.T)                     # [D, TB]
        xT_hi, xT_lo = _split_hi_lo(xT)
        m = {"xT_hi": xT_hi, "xT_lo": xT_lo,
             "x_tok": np.ascontiguousarray(own),
             "ident": np.eye(P, dtype=np.float32).astype(ml_dtypes.bfloat16),
             "ws1_hi": ws1_hi, "ws1_lo": ws1_lo}
        m.update(wsplit)
        m.update(thr)
        for nm in ("bp", "b1", "b2"):
            if cfg[f"has_{nm}"]:
                m[nm] = biases[nm]
        in_maps.append(m)
    return in_maps, cfg


_prog_cache = {}


def kernel(**inputs) -> np.ndarray:
    in_maps, cfg = make_core_inputs(**inputs)
    key = tuple(sorted(cfg.items()))
    if key not in _prog_cache:
        _prog_cache[key] = build_program(cfg)
    nc = _prog_cache[key]

    res = run_bass_kernel_spmd(nc, in_maps, core_ids=list(range(NCORES)))
    last_run_info["exec_time_ns"] = res.exec_time_ns
    last_run_info["mean_exec_time_ns"] = res.mean_exec_time_ns

    out = np.empty((B, L, D), np.float32)
    for c in range(NCORES):
        b = c // 2
        h = c % 2
        out[b, h * T:(h + 1) * T, :] = res.results[c]["out"]
    return out


# revision 40
# speedup vs baseline: 1.1452x; 1.1085x over previous
"""Trainium2 Bass kernel for nn_Block_80041010528755 (spiking transformer block).

Math structure (see reference):
  q = spike(LN(x@q_w) >= 2), k/v likewise (binary {0,1})
  attn has NO softmax -> (q@k^T)@v == q@(k^T@v): per-head 64x64 kv matrix,
  exact in bf16/fp32 because spikes are binary and sums are small integers.
  y2 = spike(LN(yspike@proj_w + pb) >= 2); x' = x + y2
  m1 = spike(LN(x'@fc1_w + b1) >= 2); m2 = spike(LN(m1@fc2_w + b2) >= 2)
  out = x' + m2

Precision: fp32-input matmuls (q/k/v from x, fc1 from x') use 3-product
bf16 hi/lo splits (x_hi@W_hi + x_lo@W_hi + x_hi@W_lo, ~2^-16 rel);
binary-input matmuls (proj, fc2) use 2 products (S@W_hi + S@W_lo).
All accumulate in fp32 PSUM.

Sharding: 8-way token-parallel, 512 tokens/core (half a batch). k/v are
computed over the core's full 1024-token batch (duplicated within the
core pair) so attention needs no collectives.
"""

import os
import sys

for _p in ("/root/.axon_site/_ro/trn_rl_repo", "/opt/trn_rl_repo"):
    if os.path.isdir(_p) and _p not in sys.path:
        sys.path.append(_p)

import numpy as np
import ml_dtypes

import concourse.bass as bass
import concourse.bacc as bacc
import concourse.tile as tile
import concourse.mybir as mybir
from concourse.bass import ts
from concourse.bass_utils import run_bass_kernel_spmd

F32 = mybir.dt.float32
BF16 = mybir.dt.bfloat16
OP = mybir.AluOpType

B, L, D = 4, 1024, 1024
HID = 4096
H, HD = 16, 64
NCORES = 8
T = 512          # own tokens per core
TB = 1024        # batch tokens per core (own + partner half)
P = 128
LN_EPS = 1e-5
THETA = 2.0      # LN-spike threshold: TAU*v_th = 2*1
ATTN_THETA = 1.0  # attn spike: y >= TAU*0.5

# module-global stash for timing info from the last kernel() call
last_run_info = {}


def _split_hi_lo(a32):
    hi = a32.astype(ml_dtypes.bfloat16)
    lo = (a32 - hi.astype(np.float32)).astype(ml_dtypes.bfloat16)
    return np.ascontiguousarray(hi), np.ascontiguousarray(lo)


def _bcast_ap(dram_ap, parts=P):
    """[D] dram tensor viewed as [parts, D] with 0-stride partitions."""
    return bass.AP(tensor=dram_ap.tensor, offset=dram_ap.offset,
                   ap=[[0, parts]] + list(dram_ap.ap))


def build_program(cfg, debug_outputs=False):
    """cfg: dict with has_bias flags + g-sign modes per LN stage."""
    nc = bacc.Bacc("TRN2", target_bir_lowering=False, debug=False)

    # ---- DRAM tensors ----
    TQKD = T if cfg["use_cc"] else TB
    xT_hi = nc.dram_tensor("xT_hi", [D, TQKD], BF16, kind="ExternalInput")
    xT_lo = nc.dram_tensor("xT_lo", [D, TQKD], BF16, kind="ExternalInput")
    x_tok = nc.dram_tensor("x_tok", [T, D], F32, kind="ExternalInput")

    w_names = {}
    for nm, (din, dout) in (("qw", (D, D)), ("kw", (D, D)), ("vw", (D, D)),
                            ("pw", (D, D)), ("f1", (D, HID)), ("f2", (HID, D))):
        for h in ("hi", "lo"):
            w_names[f"{nm}_{h}"] = nc.dram_tensor(
                f"{nm}_{h}", [din, dout], BF16, kind="ExternalInput")

    thr_names = {}
    for nm, dd in (("tq", D), ("tk", D), ("tv", D), ("tp", D),
                   ("t1", HID), ("t2", D)):
        thr_names[nm] = nc.dram_tensor(nm, [dd], F32, kind="ExternalInput")

    ident_in = nc.dram_tensor("ident", [P, P], BF16, kind="ExternalInput")
    ws1_hi = nc.dram_tensor("ws1_hi", [D], BF16, kind="ExternalInput")
    ws1_lo = nc.dram_tensor("ws1_lo", [D], BF16, kind="ExternalInput")

    bias_names = {}
    for nm, dd in (("bp", D), ("b1", HID), ("b2", D)):
        if cfg[f"has_{nm}"]:
            bias_names[nm] = nc.dram_tensor(nm, [dd], F32, kind="ExternalInput")

    out_dram = nc.dram_tensor("out", [T, D], F32, kind="ExternalOutput")

    dbg = {}
    if debug_outputs:
        TKV = T if cfg["use_cc"] else TB
        for nm, shp, dt in (("d_qsT", [D, T], BF16), ("d_ks", [TKV, D], BF16),
                            ("d_vs", [TKV, D], BF16), ("d_ysT", [D, T], BF16),
                            ("d_y2", [T, D], BF16), ("d_m1T", [HID, T], BF16),
                            ("d_z1T", [HID, T], F32)):
            dbg[nm] = nc.dram_tensor(nm, shp, dt, kind="ExternalOutput")

    # weight dram views [p, kc, dout]
    wv = {k: v.ap().rearrange("(kc p) f -> p kc f", p=P)
          for k, v in w_names.items()}

    def dbg_copy(dram, sb, fm=False):
        pat = "(c p) t -> p c t" if fm else "(c p) f -> p c f"
        dv = dram.ap().rearrange(pat, p=P)
        for c in range(sb.shape[1]):
            nc.sync.dma_start(dv[:, c, :], sb[:, c, :])

    with tile.TileContext(nc) as tc:
        with tc.tile_pool(name="psum", bufs=6, space="PSUM") as psum, \
             tc.tile_pool(name="stats", bufs=6) as stats, \
             tc.tile_pool(name="thrp", bufs=3) as thrp, \
             tc.tile_pool(name="consts", bufs=1) as consts, \
             tc.tile_pool(name="resid", bufs=1) as resid:

            eps_t = consts.tile([P, 1], F32)
            nc.vector.memset(eps_t, LN_EPS)
            ident = consts.tile([P, P], BF16, tag="ident")
            nc.sync.dma_start(ident, ident_in.ap())
            ones0 = consts.tile([P, 1], F32, tag="ones0")
            nc.vector.memset(ones0, 1.0)
            ones_r = consts.tile([P, 1], mybir.dt.float32r, tag="ones_r")
            nc.vector.tensor_copy(ones_r, ones0)

            # round-robin 128x128 DMA transposes over the two HWDGE queues
            _tp_state = [0]

            def tpose(dst, src):
                eng = (nc.sync, nc.sync)[_tp_state[0] % 2]
                _tp_state[0] += 1
                eng.dma_start(out=dst, in_=src, transpose=True)

            def load_bcast(name, dd, pool):
                t = pool.tile([P, dd], F32, tag=f"bc_{name}", name=f"bc_{name}")
                nc.gpsimd.dma_start(t, _bcast_ap(thr_names[name].ap()
                                                 if name in thr_names
                                                 else bias_names[name].ap()))
                return t

            # ---------- LN + spike helper ----------
            def ln_spike(z_chunks, thr_b, mode, out_fn, stat_tag):
                """z_chunks: APs [128, 512] covering the feature dim for one
                token block. thr_b: [128, d_total] bcast of (theta-b)/g.
                Emits spike = cmp(z, mean + t*std) per chunk via out_fn."""
                nchunks = len(z_chunks)
                st = stats.tile([P, nchunks, 6], F32, tag=f"st_{stat_tag}",
                                name=f"st_{stat_tag}")
                for j, zc in enumerate(z_chunks):
                    nc.vector.bn_stats(st[:, j], zc)
                mv = stats.tile([P, 2], F32, tag=f"mv_{stat_tag}",
                                name=f"mv_{stat_tag}")
                nc.vector.bn_aggr(mv, st)
                std = stats.tile([P, 1], F32, tag=f"sd_{stat_tag}",
                                 name=f"sd_{stat_tag}")
                nc.scalar.activation(out=std, in_=mv[:, 1:2],
                                     func=mybir.ActivationFunctionType.Sqrt,
                                     bias=eps_t, scale=1.0)
                cmp = OP.is_ge if mode == "pos" else OP.is_le
                for j, zc in enumerate(z_chunks):
                    thr = thrp.tile([P, 512], F32, tag="thr", name=f"th_{stat_tag}_{j}")
                    nc.vector.tensor_scalar(out=thr, in0=thr_b[:, ts(j, 512)],
                                            scalar1=std, scalar2=mv[:, 0:1],
                                            op0=OP.mult, op1=OP.add)
                    out_fn(j, zc, thr, cmp)

            xnew = resid.tile([P, T // P, D], F32, tag="xnew")

            with tc.tile_pool(name="xtp", bufs=1) as xtp:
                xt = xtp.tile([P, T // P, D], F32, tag="xt")
                # off the sync queue: only needed at proj time
                nc.gpsimd.dma_start(xt, x_tok.ap().rearrange("(c p) f -> p c f",
                                                             p=P))

                with tc.tile_pool(name="sp3", bufs=1) as sp3:
                    ysT = sp3.tile([P, D // P, T], BF16, tag="ysT")
                    y2 = sp3.tile([P, T // P, D], BF16, tag="y2")

                    with tc.tile_pool(name="sp12", bufs=1) as sp12:
                        TQK = T if cfg["use_cc"] else TB   # k/v token span
                        NTKV = TQK // P
                        kS = sp12.tile([P, NTKV, D], BF16, tag="kS")
                        vS = sp12.tile([P, NTKV, D], BF16, tag="vS")
                        qTS = sp12.tile([P, D // P, T], BF16, tag="qTS")
                        kvred = sp12.tile([P, D // P, P], F32, tag="kvred")

                        # ======== stage 1+2: k, v, q + kv collective =======
                        with tc.tile_pool(name="xTp", bufs=1) as xTpool, \
                             tc.tile_pool(name="tqkv", bufs=1) as tpool, \
                             tc.tile_pool(name="qsc", bufs=3) as qscp, \
                             tc.tile_pool(name="ccdram", bufs=1,
                                          space="DRAM") as ccd, \
                             tc.tile_pool(name="wqkvh", bufs=2) as wqkvh, \
                             tc.tile_pool(name="wqkvl", bufs=1) as wqkvl:
                            xTh = xTpool.tile([P, D // P, TQK], BF16, tag="xTh")
                            xTl = xTpool.tile([P, D // P, TQK], BF16, tag="xTl")
                            xThd = xT_hi.ap().rearrange("(c p) t -> p c t", p=P)
                            xTld = xT_lo.ap().rearrange("(c p) t -> p c t", p=P)
                            nc.sync.dma_start(xTh[:, :, 0:T], xThd[:, :, 0:T])
                            nc.sync.dma_start(xTl[:, :, 0:T], xTld[:, :, 0:T])
                            tq_b = load_bcast("tq", D, tpool)
                            tk_b = load_bcast("tk", D, tpool)
                            tv_b = load_bcast("tv", D, tpool)

                            if cfg["use_cc"]:
                                order = (("kw", kS, tk_b, NTKV, cfg["mode_k"]),
                                         ("vw", vS, tv_b, NTKV, cfg["mode_v"]),
                                         ("qw", None, tq_b, T // P,
                                          cfg["mode_q"]))
                            else:
                                order = (("qw", None, tq_b, T // P,
                                          cfg["mode_q"]),
                                         ("kw", kS, tk_b, NTKV, cfg["mode_k"]),
                                         ("vw", vS, tv_b, NTKV, cfg["mode_v"]))

                            for nm, spk, thr_b, ntt, mode in order:
                                if True:
                                    whi = wqkvh.tile([P, D // P, D], BF16,
                                                     tag="wqkv_hi",
                                                     name=f"{nm}_hi_t")
                                    wlo = wqkvl.tile([P, D // P, D], BF16,
                                                     tag="wqkv_lo",
                                                     name=f"{nm}_lo_t")
                                    nc.sync.dma_start(whi, wv[f"{nm}_hi"])
                                    nc.sync.dma_start(wlo, wv[f"{nm}_lo"])
                                    if not cfg["use_cc"] and nm == "qw":
                                        # partner xT halves: needed from k on
                                        nc.sync.dma_start(xTh[:, :, T:TB],
                                                          xThd[:, :, T:TB])
                                        nc.sync.dma_start(xTl[:, :, T:TB],
                                                          xTld[:, :, T:TB])
                                    for tt in range(ntt):
                                        pss = []
                                        for n in range(D // 512):
                                            ps = psum.tile([P, 512], F32, tag="mm",
                                                           name=f"ps_{nm}_{tt}_{n}")
                                            first = True
                                            for xa, wa in ((xTh, whi), (xTl, whi),
                                                           (xTh, wlo)):
                                                for kk in range(D // P):
                                                    nc.tensor.matmul(
                                                        ps, xa[:, kk, ts(tt, P)],
                                                        wa[:, kk, ts(n, 512)],
                                                        start=first,
                                                        stop=(xa is xTh and
                                                              wa is wlo and
                                                              kk == D // P - 1))
                                                    first = False
                                            pss.append(ps)

                                        if spk is None:
                                            # q: emit to scratch, PE-transpose
                                            # to feature-major immediately
                                            def emit(j, zc, thc, cmp, tt=tt):
                                                qc = qscp.tile(
                                                    [P, 512], BF16, tag="qc",
                                                    name=f"qc_{tt}_{j}")
                                                nc.vector.tensor_tensor(
                                                    out=qc, in0=zc, in1=thc,
                                                    op=cmp)
                                                for j2 in range(4):
                                                    fcx = j * 4 + j2
                                                    pt = psum.tile(
                                                        [P, P], BF16, tag="mm",
                                                        name=f"qpt_{tt}_{fcx}")
                                                    nc.tensor.transpose(
                                                        pt, qc[:, ts(j2, P)],
                                                        ident)
                                                    nc.vector.tensor_copy(
                                                        qTS[:, fcx, ts(tt, P)],
                                                        pt)
                                        else:
                                            def emit(j, zc, thc, cmp, spk=spk,
                                                     tt=tt):
                                                nc.vector.tensor_tensor(
                                                    out=spk[:, tt, ts(j, 512)],
                                                    in0=zc, in1=thc, op=cmp)
                                        ln_spike(pss, thr_b, mode, emit, "qkv")

                                if cfg["use_cc"] and nm == "vw":
                                    # kv partials + pairwise all-reduce;
                                    # latency hides under the q stage
                                    kvall = xTpool.tile([P, D // P, P], F32,
                                                        tag="kvall")
                                    for hp in range(D // P):
                                        pkv = psum.tile([P, P], F32, tag="mm",
                                                        name=f"pkv_{hp}")
                                        for tt in range(NTKV):
                                            nc.tensor.matmul(
                                                pkv, kS[:, tt, ts(hp, P)],
                                                vS[:, tt, ts(hp, P)],
                                                start=(tt == 0),
                                                stop=(tt == NTKV - 1))
                                        nc.vector.tensor_copy(kvall[:, hp], pkv)
                                    cc_in = ccd.tile([P, D], F32, tag="cc_in")
                                    cc_out = ccd.tile([P, D], F32, tag="cc_out")
                                    nc.gpsimd.dma_start(
                                        cc_in, kvall.rearrange("p c q -> p (c q)"))
                                    pair = [[2 * i, 2 * i + 1]
                                            for i in range(NCORES // 2)]
                                    nc.gpsimd.collective_compute(
                                        "AllReduce", OP.add,
                                        replica_groups=pair,
                                        ins=[cc_in.opt()], outs=[cc_out.opt()])
                                    nc.gpsimd.dma_start(
                                        kvred.rearrange("p c q -> p (c q)"),
                                        cc_out)

                            if not cfg["use_cc"]:
                                # kv straight from local psums
                                for hp in range(D // P):
                                    pkv = psum.tile([P, P], F32, tag="mm",
                                                    name=f"pkv_{hp}")
                                    for tt in range(NTKV):
                                        nc.tensor.matmul(
                                            pkv, kS[:, tt, ts(hp, P)],
                                            vS[:, tt, ts(hp, P)],
                                            start=(tt == 0),
                                            stop=(tt == NTKV - 1))
                                    nc.vector.tensor_copy(kvred[:, hp], pkv)

                        if debug_outputs:
                            dbg_copy(dbg["d_qsT"], qTS, fm=True)
                            dbg_copy(dbg["d_ks"], kS)
                            dbg_copy(dbg["d_vs"], vS)

                        # ======== stage 3: y + attn spike ==================
                        with tc.tile_pool(name="attn", bufs=4) as apool:
                            for hp in range(D // P):   # 8 head pairs
                                kvd = apool.tile([P, P], F32, tag="kvd",
                                                 name=f"kvd_{hp}")
                                nc.vector.memset(kvd, 0.0)
                                nc.vector.tensor_scalar_mul(
                                    kvd[0:HD, 0:HD], kvred[0:HD, hp, 0:HD],
                                    0.125)
                                nc.vector.tensor_scalar_mul(
                                    kvd[HD:P, HD:P], kvred[HD:P, hp, HD:P],
                                    0.125)
                                kvh = apool.tile([P, P], BF16, tag="kvh",
                                                 name=f"kvh_{hp}")
                                nc.vector.tensor_copy(kvh, kvd)
                                kvhf = apool.tile([P, P], F32, tag="kvhf",
                                                  name=f"kvhf_{hp}")
                                nc.vector.tensor_copy(kvhf, kvh)
                                kvl = apool.tile([P, P], BF16, tag="kvl",
                                                 name=f"kvl_{hp}")
                                nc.vector.tensor_tensor(out=kvl, in0=kvd,
                                                        in1=kvhf, op=OP.subtract)
                                py = psum.tile([P, T], F32, tag="mm",
                                               name=f"py_{hp}")
                                nc.tensor.matmul(py, kvh, qTS[:, hp, :],
                                                 start=True, stop=False)
                                nc.tensor.matmul(py, kvl, qTS[:, hp, :],
                                                 start=False, stop=True)
                                nc.vector.tensor_scalar(out=ysT[:, hp, :],
                                                        in0=py,
                                                        scalar1=ATTN_THETA,
                                                        scalar2=None,
                                                        op0=OP.is_ge)
                    # sp12 closed: qS/kS/vS/qTS freed

                    if debug_outputs:
                        dbg_copy(dbg["d_ysT"], ysT, fm=True)

                    # ======== stage 4: proj + LN + spike, residual =========
                    with tc.tile_pool(name="wp", bufs=1) as wpool, \
                         tc.tile_pool(name="tproj", bufs=1) as tpool, \
                         tc.tile_pool(name="zproj", bufs=4) as zpool:
                        tp_b = load_bcast("tp", D, tpool)
                        bp_b = load_bcast("bp", D, tpool) if cfg["has_bp"] else None
                        pwh = wpool.tile([P, D // P, D], BF16, tag="w_pw_hi")
                        pwl = wpool.tile([P, D // P, D], BF16, tag="w_pw_lo")
                        nc.sync.dma_start(pwh, wv["pw_hi"])
                        nc.sync.dma_start(pwl, wv["pw_lo"])
                        for tt in range(T // P):
                            zrefs = []
                            for n in range(D // 512):
                                ps = psum.tile([P, 512], F32, tag="mm",
                                               name=f"ps_pr_{tt}_{n}")
                                first = True
                                for wa in (pwh, pwl):
                                    for kk in range(D // P):
                                        nc.tensor.matmul(
                                            ps, ysT[:, kk, ts(tt, P)],
                                            wa[:, kk, ts(n, 512)],
                                            start=first,
                                            stop=(wa is pwl and kk == D // P - 1))
                                        first = False
                                if bp_b is not None:
                                    zc = zpool.tile([P, 512], F32, tag="zproj",
                                                    name=f"zpr_{tt}_{n}")
                                    nc.vector.tensor_tensor(
                                        out=zc, in0=ps,
                                        in1=bp_b[:, ts(n, 512)], op=OP.add)
                                    zrefs.append(zc)
                                else:
                                    zrefs.append(ps)

                            def emit(j, zc, thc, cmp, tt=tt):
                                nc.vector.tensor_tensor(
                                    out=y2[:, tt, ts(j, 512)],
                                    in0=zc, in1=thc, op=cmp)
                            ln_spike(zrefs, tp_b, cfg["mode_p"], emit, "proj")
                            nc.vector.tensor_tensor(out=xnew[:, tt, :],
                                                    in0=xt[:, tt, :],
                                                    in1=y2[:, tt, :], op=OP.add)

                    if debug_outputs:
                        dbg_copy(dbg["d_y2"], y2)
                # sp3 closed: ysT, y2 freed
            # xtp closed: xt freed

            # ============ stage 5+6+7 ======================================
            with tc.tile_pool(name="sp6", bufs=1) as sp6:
                m1T = sp6.tile([P, HID // P, T], BF16, tag="m1T")

                with tc.tile_pool(name="sp5", bufs=1) as sp5:
                    xnT_h = sp5.tile([P, D // P, T], BF16, tag="xnT_h")
                    xnT_l = sp5.tile([P, D // P, T], BF16, tag="xnT_l")
                    # ---- stage 5: split xnew + PE transpose ----
                    with tc.tile_pool(name="xsplit", bufs=3) as xsp:
                        for tt in range(T // P):
                            xh = xsp.tile([P, D], BF16, tag="xh", name=f"xh_{tt}")
                            xl = xsp.tile([P, D], BF16, tag="xl", name=f"xl_{tt}")
                            xhf = xsp.tile([P, D], F32, tag="xhf", name=f"xhf_{tt}")
                            nc.vector.tensor_copy(xh, xnew[:, tt, :])
                            nc.vector.tensor_copy(xhf, xh)
                            nc.vector.tensor_tensor(out=xl, in0=xnew[:, tt, :],
                                                    in1=xhf, op=OP.subtract)
                            for fc in range(D // P):
                                for src, dst in ((xh, xnT_h), (xl, xnT_l)):
                                    pt = psum.tile([P, P], BF16, tag="mm",
                                                   name=f"pt_{tt}_{fc}")
                                    nc.tensor.transpose(pt, src[:, ts(fc, P)],
                                                        ident)
                                    nc.vector.tensor_copy(
                                        dst[:, fc, ts(tt, P)], pt)

                    # ---- stage 6: fc1, FEATURE-major so m1 spikes land
                    #      directly in fc2's lhsT layout (no transposes).
                    #      LN stats via fp32r ones-matmul reductions. ----
                    F32R = mybir.dt.float32r
                    NMC = HID // P   # 32 dout chunks
                    with tc.tile_pool(name="z1p", bufs=1) as z1pool, \
                         tc.tile_pool(name="tfc1", bufs=1) as tpool, \
                         tc.tile_pool(name="wf1", bufs=2) as wpool, \
                         tc.tile_pool(name="psred", bufs=1, space="PSUM") as psr, \
                         tc.tile_pool(name="fc1ln", bufs=4) as lp:
                        # t1 / b1 as per-partition [128, 32] (feature-major)
                        t1_fm = tpool.tile([P, NMC], F32, tag="t1_fm")
                        nc.sync.dma_start(
                            t1_fm, thr_names["t1"].ap().rearrange(
                                "(c p) -> p c", p=P))
                        b1_fm = None
                        if cfg["has_b1"]:
                            b1_fm = tpool.tile([P, NMC], F32, tag="b1_fm")
                            nc.sync.dma_start(
                                b1_fm, bias_names["b1"].ap().rearrange(
                                    "(c p) -> p c", p=P))
                        z1T = z1pool.tile([P, NMC, T], F32, tag="z1T")
                        pr_sum = psr.tile([1, T], F32, tag="pr_sum")
                        pr_sq = psr.tile([1, T], F32, tag="pr_sq")
                        cmp1 = OP.is_ge if cfg["mode_1"] == "pos" else OP.is_le

                        # mean*HID = xn @ rowsum(fc1_w) (+ sum(b1), host-folded)
                        wsh = tpool.tile([P, D // P], BF16, tag="ws1h")
                        wsl = tpool.tile([P, D // P], BF16, tag="ws1l")
                        nc.sync.dma_start(wsh, ws1_hi.ap().rearrange(
                            "(c p) -> p c", p=P))
                        nc.sync.dma_start(wsl, ws1_lo.ap().rearrange(
                            "(c p) -> p c", p=P))
                        # two token halves: half 0's LN-apply overlaps
                        # half 1's matmuls (fc1 weights streamed twice)
                        T2 = T // 2
                        for hf in range(2):
                            hsl = bass.ds(hf * T2, T2)
                            first = True
                            for xa, wa in ((xnT_h, wsh), (xnT_l, wsh),
                                           (xnT_h, wsl)):
                                for kk in range(D // P):
                                    nc.tensor.matmul(
                                        pr_sum[:, hsl], wa[:, kk:kk + 1],
                                        xa[:, kk, hsl],
                                        start=first,
                                        stop=(xa is xnT_h and wa is wsl and
                                              kk == D // P - 1))
                                    first = False

                            for mc in range(NMC):
                                if mc % 4 == 0:
                                    # batched weight load: 4 dout chunks
                                    w4h = wpool.tile([P, D // P, 4 * P], BF16,
                                                     tag="f1h",
                                                     name=f"f1h_{hf}_{mc}")
                                    w4l = wpool.tile([P, D // P, 4 * P], BF16,
                                                     tag="f1l",
                                                     name=f"f1l_{hf}_{mc}")
                                    nc.sync.dma_start(
                                        w4h, wv["f1_hi"][:, :, ts(mc // 4, 4 * P)])
                                    nc.sync.dma_start(
                                        w4l, wv["f1_lo"][:, :, ts(mc // 4, 4 * P)])
                                wh = w4h[:, :, ts(mc % 4, P)]
                                wl = w4l[:, :, ts(mc % 4, P)]
                                ps = psum.tile([P, T2], F32, tag="mm",
                                               name=f"ps_f1_{hf}_{mc}")
                                first = True
                                for xa, wa in ((xnT_h, wh), (xnT_l, wh),
                                               (xnT_h, wl)):
                                    for kk in range(D // P):
                                        nc.tensor.matmul(
                                            ps, wa[:, kk, :], xa[:, kk, hsl],
                                            start=first,
                                            stop=(xa is xnT_h and wa is wl and
                                                  kk == D // P - 1))
                                        first = False
                                if b1_fm is not None:
                                    nc.vector.tensor_scalar(
                                        out=z1T[:, mc, hsl], in0=ps,
                                        scalar1=b1_fm[:, mc:mc + 1],
                                        scalar2=None, op0=OP.add)
                                else:
                                    nc.vector.tensor_copy(z1T[:, mc, hsl], ps)
                                zq = lp.tile([P, T2], F32R, tag="zq",
                                             name=f"zq_{hf}_{mc}")
                                nc.scalar.activation(
                                    out=zq, in_=z1T[:, mc, hsl],
                                    func=mybir.ActivationFunctionType.Square,
                                    bias=0.0, scale=1.0)
                                nc.tensor.matmul(pr_sq[:, hsl], ones_r, zq,
                                                 start=(mc == 0),
                                                 stop=(mc == NMC - 1))

                            # stats for this half
                            mrow = lp.tile([1, T2], F32, tag="mrow",
                                           name=f"mrow_{hf}")
                            nc.vector.tensor_scalar(
                                out=mrow, in0=pr_sum[:, hsl],
                                scalar1=1.0 / HID, scalar2=cfg["b1_sum"] / HID,
                                op0=OP.mult, op1=OP.add)
                            e2row = lp.tile([1, T2], F32, tag="e2row",
                                            name=f"e2row_{hf}")
                            nc.vector.tensor_scalar_mul(e2row, pr_sq[:, hsl],
                                                        1.0 / HID)
                            vrow = lp.tile([1, T2], F32, tag="vrow",
                                           name=f"vrow_{hf}")
                            nc.vector.tensor_tensor(out=vrow, in0=mrow,
                                                    in1=mrow, op=OP.mult)
                            nc.vector.tensor_tensor(out=vrow, in0=e2row,
                                                    in1=vrow, op=OP.subtract)
                            srow = lp.tile([1, T2], F32, tag="srow",
                                           name=f"srow_{hf}")
                            nc.scalar.activation(
                                out=srow, in_=vrow,
                                func=mybir.ActivationFunctionType.Sqrt,
                                bias=eps_t[0:1], scale=1.0)
                            m_b = lp.tile([P, T2], F32, tag="m_b",
                                          name=f"m_b_{hf}")
                            s_b = lp.tile([P, T2], F32, tag="s_b",
                                          name=f"s_b_{hf}")
                            nc.gpsimd.partition_broadcast(m_b, mrow)
                            nc.gpsimd.partition_broadcast(s_b, srow)
                            for mc in range(NMC):
                                thr = thrp.tile([P, T2], F32, tag="thr",
                                                name=f"th1_{hf}_{mc}")
                                nc.vector.tensor_scalar(
                                    out=thr, in0=s_b,
                                    scalar1=t1_fm[:, mc:mc + 1], scalar2=None,
                                    op0=OP.mult)
                                nc.vector.tensor_tensor(out=thr, in0=thr,
                                                        in1=m_b, op=OP.add)
                                nc.vector.tensor_tensor(
                                    out=m1T[:, mc, hsl], in0=z1T[:, mc, hsl],
                                    in1=thr, op=cmp1)

                        if debug_outputs:
                            dbg_copy(dbg["d_z1T"], z1T, fm=True)
                # sp5 closed: xnT freed

                if debug_outputs:
                    dbg_copy(dbg["d_m1T"], m1T, fm=True)

                # ---- stage 7: fc2 ----
                with tc.tile_pool(name="z2p", bufs=1) as z2pool, \
                     tc.tile_pool(name="tfc2", bufs=1) as tpool, \
                     tc.tile_pool(name="wf2", bufs=3) as wpool, \
                     tc.tile_pool(name="fc2ln", bufs=3) as lp:
                    t2_b = load_bcast("t2", D, tpool)
                    b2_b = load_bcast("b2", D, tpool) if cfg["has_b2"] else None
                    z2 = z2pool.tile([P, T // P, D], F32, tag="z2")
                    st2 = z2pool.tile([P, T // P, D // 512, 6], F32, tag="st_fc2")
                    for n in range(D // 512):
                        pss = []
                        for _pi in range(T // P):
                            pst = psum.tile([P, 512], F32, tag="mm",
                                            name=f"ps2_{n}_{_pi}")
                            pss.append(pst)
                        for kk in range(HID // P):
                            if kk % 4 == 0:
                                w4h = wpool.tile([P, 4, 512], BF16, tag="f2h",
                                                 name=f"f2h_{n}_{kk}")
                                w4l = wpool.tile([P, 4, 512], BF16, tag="f2l",
                                                 name=f"f2l_{n}_{kk}")
                                nc.sync.dma_start(
                                    w4h, wv["f2_hi"][:, bass.ds(kk, 4),
                                                     ts(n, 512)])
                                nc.sync.dma_start(
                                    w4l, wv["f2_lo"][:, bass.ds(kk, 4),
                                                     ts(n, 512)])
                            wh = w4h[:, kk % 4]
                            wl = w4l[:, kk % 4]
                            for tt in range(T // P):
                                nc.tensor.matmul(pss[tt], m1T[:, kk, ts(tt, P)],
                                                 wh, start=(kk == 0), stop=False)
                                nc.tensor.matmul(pss[tt], m1T[:, kk, ts(tt, P)],
                                                 wl, start=False,
                                                 stop=(kk == HID // P - 1))
                        for tt in range(T // P):
                            if b2_b is not None:
                                nc.vector.tensor_tensor(
                                    out=z2[:, tt, ts(n, 512)], in0=pss[tt],
                                    in1=b2_b[:, ts(n, 512)], op=OP.add)
                            else:
                                nc.vector.tensor_copy(z2[:, tt, ts(n, 512)],
                                                      pss[tt])
                            nc.vector.bn_stats(st2[:, tt, n],
                                               z2[:, tt, ts(n, 512)])

                    cmp2 = OP.is_ge if cfg["mode_2"] == "pos" else OP.is_le
                    for tt in range(T // P):
                        mv = lp.tile([P, 2], F32, tag="mv2", name=f"mv2_{tt}")
                        nc.vector.bn_aggr(mv, st2[:, tt])
                        std = lp.tile([P, 1], F32, tag="sd2", name=f"sd2_{tt}")
                        nc.scalar.activation(
                            out=std, in_=mv[:, 1:2],
                            func=mybir.ActivationFunctionType.Sqrt,
                            bias=eps_t, scale=1.0)
                        for n in range(D // 512):
                            thr = thrp.tile([P, 512], F32, tag="thr",
                                            name=f"th2_{tt}_{n}")
                            nc.vector.tensor_scalar(
                                out=thr, in0=t2_b[:, ts(n, 512)],
                                scalar1=std, scalar2=mv[:, 0:1],
                                op0=OP.mult, op1=OP.add)
                            m2c = lp.tile([P, 512], F32, tag="m2c",
                                          name=f"m2c_{tt}_{n}")
                            nc.vector.tensor_tensor(
                                out=m2c, in0=z2[:, tt, ts(n, 512)],
                                in1=thr, op=cmp2)
                            ot = lp.tile([P, 512], F32, tag="ot",
                                         name=f"ot_{tt}_{n}")
                            nc.vector.tensor_tensor(
                                out=ot, in0=xnew[:, tt, ts(n, 512)],
                                in1=m2c, op=OP.add)
                            nc.sync.dma_start(
                                out_dram.ap().rearrange(
                                    "(c p) f -> p c f", p=P)[:, tt, ts(n, 512)],
                                ot)

    nc.compile()
    return nc


def _sign_mode(g):
    if np.all(g > 0):
        return "pos"
    if np.all(g < 0):
        return "neg"
    raise NotImplementedError("mixed-sign LN gain not supported")


def make_core_inputs(x, q_w, q_g, q_b, k_w, k_g, k_b, v_w, v_g, v_b,
                     proj_w, proj_bias, proj_g, proj_beta,
                     fc1_w, fc1_bias, fc1_g, fc1_beta,
                     fc2_w, fc2_bias, fc2_g, fc2_beta):
    f32 = np.float32
    X = np.asarray(x, f32).reshape(B * L, D)

    wsplit = {}
    for nm, W in (("qw", q_w), ("kw", k_w), ("vw", v_w),
                  ("pw", proj_w), ("f1", fc1_w), ("f2", fc2_w)):
        hi, lo = _split_hi_lo(np.asarray(W, f32))
        wsplit[f"{nm}_hi"] = hi
        wsplit[f"{nm}_lo"] = lo

    def thrvec(g, b):
        return ((THETA - np.asarray(b, np.float64))
                / np.asarray(g, np.float64)).astype(f32)

    thr = {"tq": thrvec(q_g, q_b), "tk": thrvec(k_g, k_b),
           "tv": thrvec(v_g, v_b), "tp": thrvec(proj_g, proj_beta),
           "t1": thrvec(fc1_g, fc1_beta), "t2": thrvec(fc2_g, fc2_beta)}

    ws1 = np.asarray(fc1_w, np.float64).sum(axis=1).astype(f32)
    ws1_hi, ws1_lo = _split_hi_lo(ws1)
    cfg = {
        "use_cc": os.environ.get("KERNEL_NO_CC", "0") != "1",
        "b1_sum": float(np.asarray(fc1_bias, np.float64).sum()),
        "mode_q": _sign_mode(np.asarray(q_g)), "mode_k": _sign_mode(np.asarray(k_g)),
        "mode_v": _sign_mode(np.asarray(v_g)), "mode_p": _sign_mode(np.asarray(proj_g)),
        "mode_1": _sign_mode(np.asarray(fc1_g)), "mode_2": _sign_mode(np.asarray(fc2_g)),
        "has_bp": bool(np.any(np.asarray(proj_bias) != 0)),
        "has_b1": bool(np.any(np.asarray(fc1_bias) != 0)),
        "has_b2": bool(np.any(np.asarray(fc2_bias) != 0)),
    }
    biases = {"bp": np.asarray(proj_bias, f32), "b1": np.asarray(fc1_bias, f32),
              "b2": np.asarray(fc2_bias, f32)}

    use_cc = cfg["use_cc"]
    in_maps = []
    for c in range(NCORES):
        b = c // 2
        h = c % 2
        own = X[b * L + h * T: b * L + (h + 1) * T]
        if use_cc:
            xT = np.ascontiguousarray(own.T)                # [D, T]
        else:
            other = X[b * L + (1 - h) * T: b * L + (2 - h) * T]
            Xp = np.concatenate([own, other], axis=0)       # [TB, D] own-first
            xT = np.ascontiguousarray(Xp.T)                 # [D, TB]
        xT_hi, xT_lo = _split_hi_lo(xT)
        m = {"xT_hi": xT_hi, "xT_lo": xT_lo,
             "x_tok": np.ascontiguousarray(own),
             "ident": np.eye(P, dtype=np.float32).astype(ml_dtypes.bfloat16),
             "ws1_hi": ws1_hi, "ws1_lo": ws1_lo}
        m.update(wsplit)
        m.update(thr)
        for nm in ("bp", "b1", "b2"):
            if cfg[f"has_{nm}"]:
                m[nm] = biases[nm]
        in_maps.append(m)
    return in_maps, cfg


_prog_cache = {}


def kernel(**inputs) -> np.ndarray:
    in_maps, cfg = make_core_inputs(**inputs)
    key = tuple(sorted(cfg.items()))
    if key not in _prog_cache:
        _prog_cache[key] = build_program(cfg)
    nc = _prog_cache[key]

    res = run_bass_kernel_spmd(nc, in_maps, core_ids=list(range(NCORES)))
    last_run_info["exec_time_ns"] = res.exec_time_ns
    last_run_info["mean_exec_time_ns"] = res.mean_exec_time_ns

    out = np.empty((B, L, D), np.float32)
    for c in range(NCORES):
        b = c // 2
        h = c % 2
        out[b, h * T:(h + 1) * T, :] = res.results[c]["out"]
    return out


# revision 48
# speedup vs baseline: 1.2994x; 1.1347x over previous
"""Trainium2 Bass kernel for nn_Block_80041010528755 (spiking transformer block).

Math structure (see reference):
  q = spike(LN(x@q_w) >= 2), k/v likewise (binary {0,1})
  attn has NO softmax -> (q@k^T)@v == q@(k^T@v): per-head 64x64 kv matrix,
  exact in bf16/fp32 because spikes are binary and sums are small integers.
  y2 = spike(LN(yspike@proj_w + pb) >= 2); x' = x + y2
  m1 = spike(LN(x'@fc1_w + b1) >= 2); m2 = spike(LN(m1@fc2_w + b2) >= 2)
  out = x' + m2

Precision: fp32-input matmuls (q/k/v from x, fc1 from x') use 3-product
bf16 hi/lo splits (x_hi@W_hi + x_lo@W_hi + x_hi@W_lo, ~2^-16 rel);
binary-input matmuls (proj, fc2) use 2 products (S@W_hi + S@W_lo).
All accumulate in fp32 PSUM.

Sharding: 8-way token-parallel, 512 tokens/core (half a batch). k/v are
computed over the core's full 1024-token batch (duplicated within the
core pair) so attention needs no collectives.
"""

import os
import sys

for _p in ("/root/.axon_site/_ro/trn_rl_repo", "/opt/trn_rl_repo"):
    if os.path.isdir(_p) and _p not in sys.path:
        sys.path.append(_p)

import numpy as np
import ml_dtypes

import concourse.bass as bass
import concourse.bacc as bacc
import concourse.tile as tile
import concourse.mybir as mybir
from concourse.bass import ts
from concourse.bass_utils import run_bass_kernel_spmd

F32 = mybir.dt.float32
BF16 = mybir.dt.bfloat16
OP = mybir.AluOpType

B, L, D = 4, 1024, 1024
HID = 4096
H, HD = 16, 64
NCORES = 8
T = 512          # own tokens per core
TB = 1024        # batch tokens per core (own + partner half)
P = 128
LN_EPS = 1e-5
THETA = 2.0      # LN-spike threshold: TAU*v_th = 2*1
ATTN_THETA = 1.0  # attn spike: y >= TAU*0.5

# module-global stash for timing info from the last kernel() call
last_run_info = {}


def _split_hi_lo(a32):
    hi = a32.astype(ml_dtypes.bfloat16)
    lo = (a32 - hi.astype(np.float32)).astype(ml_dtypes.bfloat16)
    return np.ascontiguousarray(hi), np.ascontiguousarray(lo)


def _bcast_ap(dram_ap, parts=P):
    """[D] dram tensor viewed as [parts, D] with 0-stride partitions."""
    return bass.AP(tensor=dram_ap.tensor, offset=dram_ap.offset,
                   ap=[[0, parts]] + list(dram_ap.ap))


def build_program(cfg, debug_outputs=False):
    """cfg: dict with has_bias flags + g-sign modes per LN stage."""
    nc = bacc.Bacc("TRN2", target_bir_lowering=False, debug=False)

    # ---- DRAM tensors ----
    TQKD = T if cfg["use_cc"] else TB
    xT_hi = nc.dram_tensor("xT_hi", [D, TQKD], BF16, kind="ExternalInput")
    xT_lo = nc.dram_tensor("xT_lo", [D, TQKD], BF16, kind="ExternalInput")
    x_tok = nc.dram_tensor("x_tok", [T, D], F32, kind="ExternalInput")

    w_names = {}
    for nm, (din, dout) in (("qw", (D, D)), ("kw", (D, D)), ("vw", (D, D)),
                            ("pw", (D, D)), ("f1", (D, HID)), ("f2", (HID, D))):
        for h in ("hi", "lo"):
            w_names[f"{nm}_{h}"] = nc.dram_tensor(
                f"{nm}_{h}", [din, dout], BF16, kind="ExternalInput")

    thr_names = {}
    for nm, dd in (("tq", D), ("tk", D), ("tv", D), ("tp", D),
                   ("t1", HID), ("t2", D)):
        thr_names[nm] = nc.dram_tensor(nm, [dd], F32, kind="ExternalInput")

    ident_in = nc.dram_tensor("ident", [P, P], BF16, kind="ExternalInput")
    ws1_hi = nc.dram_tensor("ws1_hi", [D], BF16, kind="ExternalInput")
    ws1_lo = nc.dram_tensor("ws1_lo", [D], BF16, kind="ExternalInput")

    bias_names = {}
    for nm, dd in (("bp", D), ("b1", HID), ("b2", D)):
        if cfg[f"has_{nm}"]:
            bias_names[nm] = nc.dram_tensor(nm, [dd], F32, kind="ExternalInput")

    out_dram = nc.dram_tensor("out", [T, D], F32, kind="ExternalOutput")

    dbg = {}
    if debug_outputs:
        TKV = T if cfg["use_cc"] else TB
        for nm, shp, dt in (("d_qsT", [D, T], BF16), ("d_ks", [TKV, D], BF16),
                            ("d_vs", [TKV, D], BF16), ("d_ysT", [D, T], BF16),
                            ("d_y2", [T, D], BF16), ("d_m1T", [HID, T], BF16),
                            ("d_z1T", [HID, T], F32)):
            dbg[nm] = nc.dram_tensor(nm, shp, dt, kind="ExternalOutput")

    # weight dram views [p, kc, dout]
    wv = {k: v.ap().rearrange("(kc p) f -> p kc f", p=P)
          for k, v in w_names.items()}

    def dbg_copy(dram, sb, fm=False):
        pat = "(c p) t -> p c t" if fm else "(c p) f -> p c f"
        dv = dram.ap().rearrange(pat, p=P)
        for c in range(sb.shape[1]):
            nc.sync.dma_start(dv[:, c, :], sb[:, c, :])

    with tile.TileContext(nc) as tc:
        with tc.tile_pool(name="psum", bufs=6, space="PSUM") as psum, \
             tc.tile_pool(name="stats", bufs=6) as stats, \
             tc.tile_pool(name="thrp", bufs=3) as thrp, \
             tc.tile_pool(name="consts", bufs=1) as consts, \
             tc.tile_pool(name="resid", bufs=1) as resid:

            eps_t = consts.tile([P, 1], F32)
            nc.vector.memset(eps_t, LN_EPS)
            ident = consts.tile([P, P], BF16, tag="ident")
            nc.sync.dma_start(ident, ident_in.ap())
            ones0 = consts.tile([P, 1], F32, tag="ones0")
            nc.vector.memset(ones0, 1.0)
            ones_r = consts.tile([P, 1], mybir.dt.float32r, tag="ones_r")
            nc.vector.tensor_copy(ones_r, ones0)

            # round-robin 128x128 DMA transposes over the two HWDGE queues
            _tp_state = [0]

            def tpose(dst, src):
                eng = (nc.sync, nc.sync)[_tp_state[0] % 2]
                _tp_state[0] += 1
                eng.dma_start(out=dst, in_=src, transpose=True)

            def load_bcast(name, dd, pool):
                t = pool.tile([P, dd], F32, tag=f"bc_{name}", name=f"bc_{name}")
                nc.gpsimd.dma_start(t, _bcast_ap(thr_names[name].ap()
                                                 if name in thr_names
                                                 else bias_names[name].ap()))
                return t

            # ---------- LN + spike helper (token-major) ----------
            def ln_spike(z_chunks, thr_b, mode, out_fn, stat_tag, tconst=None):
                """z_chunks: APs [128, 512] covering the feature dim for one
                token block. thr_b: [128, d_total] bcast of (theta-b)/g
                (unused when tconst given). Emits spike = cmp(z, mean+t*std).
                With constant t, the threshold is a per-token scalar
                thr = mean + t*std -> single tensor_scalar per chunk."""
                nchunks = len(z_chunks)
                st = stats.tile([P, nchunks, 6], F32, tag=f"st_{stat_tag}",
                                name=f"st_{stat_tag}")
                for j, zc in enumerate(z_chunks):
                    nc.vector.bn_stats(st[:, j], zc)
                mv = stats.tile([P, 2], F32, tag=f"mv_{stat_tag}",
                                name=f"mv_{stat_tag}")
                nc.vector.bn_aggr(mv, st)
                std = stats.tile([P, 1], F32, tag=f"sd_{stat_tag}",
                                 name=f"sd_{stat_tag}")
                nc.scalar.activation(out=std, in_=mv[:, 1:2],
                                     func=mybir.ActivationFunctionType.Sqrt,
                                     bias=eps_t, scale=1.0)
                cmp = OP.is_ge if mode == "pos" else OP.is_le
                if tconst is not None:
                    thrc = stats.tile([P, 1], F32, tag=f"tc_{stat_tag}",
                                      name=f"tc_{stat_tag}")
                    nc.vector.tensor_scalar(out=thrc, in0=std,
                                            scalar1=float(tconst),
                                            scalar2=mv[:, 0:1],
                                            op0=OP.mult, op1=OP.add)
                    for j, zc in enumerate(z_chunks):
                        out_fn(j, zc, thrc, cmp)
                    return
                for j, zc in enumerate(z_chunks):
                    thr = thrp.tile([P, 512], F32, tag="thr", name=f"th_{stat_tag}_{j}")
                    nc.vector.tensor_scalar(out=thr, in0=thr_b[:, ts(j, 512)],
                                            scalar1=std, scalar2=mv[:, 0:1],
                                            op0=OP.mult, op1=OP.add)
                    out_fn(j, zc, thr, cmp)

            def emit_cmp(out_ap, zc, thc, cmp):
                if thc.free_size() == 1:
                    nc.vector.tensor_scalar(out=out_ap, in0=zc, scalar1=thc,
                                            scalar2=None, op0=cmp)
                else:
                    nc.vector.tensor_tensor(out=out_ap, in0=zc, in1=thc, op=cmp)

            xnew = resid.tile([P, T // P, D], F32, tag="xnew")

            with tc.tile_pool(name="xtp", bufs=1) as xtp:
                xt = xtp.tile([P, T // P, D], F32, tag="xt")
                # off the sync queue: only needed at proj time
                nc.gpsimd.dma_start(xt, x_tok.ap().rearrange("(c p) f -> p c f",
                                                             p=P))

                with tc.tile_pool(name="sp3", bufs=1) as sp3, \
                     tc.tile_pool(name="wp", bufs=1) as wpool:
                    ysT = sp3.tile([P, D // P, T], BF16, tag="ysT")
                    y2 = sp3.tile([P, T // P, D], BF16, tag="y2")
                    # proj weights: prefetched during stages 1-3
                    pwh = wpool.tile([P, D // P, D], BF16, tag="w_pw_hi")
                    pwl = wpool.tile([P, D // P, D], BF16, tag="w_pw_lo")

                    with tc.tile_pool(name="sp12", bufs=1) as sp12:
                        TQK = T if cfg["use_cc"] else TB   # k/v token span
                        NTKV = TQK // P
                        kS = sp12.tile([P, NTKV, D], BF16, tag="kS")
                        vS = sp12.tile([P, NTKV, D], BF16, tag="vS")
                        qTS = sp12.tile([P, D // P, T], BF16, tag="qTS")
                        kvred = sp12.tile([P, D // P, P], F32, tag="kvred")

                        # ======== stage 1+2: k, v, q + kv collective =======
                        with tc.tile_pool(name="xTp", bufs=1) as xTpool, \
                             tc.tile_pool(name="tqkv", bufs=1) as tpool, \
                             tc.tile_pool(name="qsc", bufs=3) as qscp, \
                             tc.tile_pool(name="ccdram", bufs=1,
                                          space="DRAM") as ccd, \
                             tc.tile_pool(name="wqkvh", bufs=(2 if cfg["use_cc"] else 1)) as wqkvh, \
                             tc.tile_pool(name="wqkvl", bufs=1) as wqkvl:
                            xTh = xTpool.tile([P, D // P, TQK], BF16, tag="xTh")
                            xTl = xTpool.tile([P, D // P, TQK], BF16, tag="xTl")
                            xThd = xT_hi.ap().rearrange("(c p) t -> p c t", p=P)
                            xTld = xT_lo.ap().rearrange("(c p) t -> p c t", p=P)
                            nc.sync.dma_start(xTh[:, :, 0:T], xThd[:, :, 0:T])
                            nc.sync.dma_start(xTl[:, :, 0:T], xTld[:, :, 0:T])
                            tq_b = (load_bcast("tq", D, tpool)
                                    if cfg["tq_c"] is None else None)
                            tk_b = (load_bcast("tk", D, tpool)
                                    if cfg["tk_c"] is None else None)
                            tv_b = (load_bcast("tv", D, tpool)
                                    if cfg["tv_c"] is None else None)

                            if cfg["use_cc"]:
                                order = (("kw", kS, tk_b, NTKV, cfg["mode_k"]),
                                         ("vw", vS, tv_b, NTKV, cfg["mode_v"]),
                                         ("qw", None, tq_b, T // P,
                                          cfg["mode_q"]))
                            else:
                                order = (("qw", None, tq_b, T // P,
                                          cfg["mode_q"]),
                                         ("kw", kS, tk_b, NTKV, cfg["mode_k"]),
                                         ("vw", vS, tv_b, NTKV, cfg["mode_v"]))

                            for nm, spk, thr_b, ntt, mode in order:
                                if True:
                                    whi = wqkvh.tile([P, D // P, D], BF16,
                                                     tag="wqkv_hi",
                                                     name=f"{nm}_hi_t")
                                    wlo = wqkvl.tile([P, D // P, D], BF16,
                                                     tag="wqkv_lo",
                                                     name=f"{nm}_lo_t")
                                    nc.sync.dma_start(whi, wv[f"{nm}_hi"])
                                    nc.sync.dma_start(wlo, wv[f"{nm}_lo"])
                                    if nm == "qw":
                                        nc.sync.dma_start(pwh, wv["pw_hi"])
                                        nc.sync.dma_start(pwl, wv["pw_lo"])
                                    if not cfg["use_cc"] and nm == "qw":
                                        # partner xT halves: needed from k on
                                        nc.sync.dma_start(xTh[:, :, T:TB],
                                                          xThd[:, :, T:TB])
                                        nc.sync.dma_start(xTl[:, :, T:TB],
                                                          xTld[:, :, T:TB])
                                    for tt in range(ntt):
                                        pss = []
                                        for n in range(D // 512):
                                            ps = psum.tile([P, 512], F32, tag="mm",
                                                           name=f"ps_{nm}_{tt}_{n}")
                                            first = True
                                            for xa, wa in ((xTh, whi), (xTl, whi),
                                                           (xTh, wlo)):
                                                for kk in range(D // P):
                                                    nc.tensor.matmul(
                                                        ps, xa[:, kk, ts(tt, P)],
                                                        wa[:, kk, ts(n, 512)],
                                                        start=first,
                                                        stop=(xa is xTh and
                                                              wa is wlo and
                                                              kk == D // P - 1))
                                                    first = False
                                            pss.append(ps)

                                        if spk is None:
                                            # q: emit to scratch, PE-transpose
                                            # to feature-major immediately
                                            def emit(j, zc, thc, cmp, tt=tt):
                                                qc = qscp.tile(
                                                    [P, 512], BF16, tag="qc",
                                                    name=f"qc_{tt}_{j}")
                                                emit_cmp(qc, zc, thc, cmp)
                                                for j2 in range(4):
                                                    fcx = j * 4 + j2
                                                    pt = psum.tile(
                                                        [P, P], BF16, tag="mm",
                                                        name=f"qpt_{tt}_{fcx}")
                                                    nc.tensor.transpose(
                                                        pt, qc[:, ts(j2, P)],
                                                        ident)
                                                    nc.vector.tensor_copy(
                                                        qTS[:, fcx, ts(tt, P)],
                                                        pt)
                                        else:
                                            def emit(j, zc, thc, cmp, spk=spk,
                                                     tt=tt):
                                                emit_cmp(spk[:, tt, ts(j, 512)],
                                                         zc, thc, cmp)
                                        ln_spike(pss, thr_b, mode, emit, "qkv",
                                                 tconst=cfg[f"t{nm[0]}_c"])

                                if cfg["use_cc"] and nm == "vw":
                                    # kv partials + pairwise all-reduce;
                                    # latency hides under the q stage
                                    kvall = xTpool.tile([P, D // P, P], F32,
                                                        tag="kvall")
                                    for hp in range(D // P):
                                        pkv = psum.tile([P, P], F32, tag="mm",
                                                        name=f"pkv_{hp}")
                                        for tt in range(NTKV):
                                            nc.tensor.matmul(
                                                pkv, kS[:, tt, ts(hp, P)],
                                                vS[:, tt, ts(hp, P)],
                                                start=(tt == 0),
                                                stop=(tt == NTKV - 1))
                                        nc.vector.tensor_copy(kvall[:, hp], pkv)
                                    cc_in = ccd.tile([P, D], F32, tag="cc_in")
                                    cc_out = ccd.tile([P, D], F32, tag="cc_out")
                                    nc.gpsimd.dma_start(
                                        cc_in, kvall.rearrange("p c q -> p (c q)"))
                                    pair = [[2 * i, 2 * i + 1]
                                            for i in range(NCORES // 2)]
                                    nc.gpsimd.collective_compute(
                                        "AllReduce", OP.add,
                                        replica_groups=pair,
                                        ins=[cc_in.opt()], outs=[cc_out.opt()])
                                    nc.gpsimd.dma_start(
                                        kvred.rearrange("p c q -> p (c q)"),
                                        cc_out)

                            if not cfg["use_cc"]:
                                # kv straight from local psums
                                for hp in range(D // P):
                                    pkv = psum.tile([P, P], F32, tag="mm",
                                                    name=f"pkv_{hp}")
                                    for tt in range(NTKV):
                                        nc.tensor.matmul(
                                            pkv, kS[:, tt, ts(hp, P)],
                                            vS[:, tt, ts(hp, P)],
                                            start=(tt == 0),
                                            stop=(tt == NTKV - 1))
                                    nc.vector.tensor_copy(kvred[:, hp], pkv)

                        if debug_outputs:
                            dbg_copy(dbg["d_qsT"], qTS, fm=True)
                            dbg_copy(dbg["d_ks"], kS)
                            dbg_copy(dbg["d_vs"], vS)

                        # ======== stage 3: y + attn spike ==================
                        with tc.tile_pool(name="attn", bufs=4) as apool:
                            for hp in range(D // P):   # 8 head pairs
                                kvd = apool.tile([P, P], F32, tag="kvd",
                                                 name=f"kvd_{hp}")
                                nc.vector.memset(kvd, 0.0)
                                nc.vector.tensor_scalar_mul(
                                    kvd[0:HD, 0:HD], kvred[0:HD, hp, 0:HD],
                                    0.125)
                                nc.vector.tensor_scalar_mul(
                                    kvd[HD:P, HD:P], kvred[HD:P, hp, HD:P],
                                    0.125)
                                kvh = apool.tile([P, P], BF16, tag="kvh",
                                                 name=f"kvh_{hp}")
                                nc.vector.tensor_copy(kvh, kvd)
                                kvhf = apool.tile([P, P], F32, tag="kvhf",
                                                  name=f"kvhf_{hp}")
                                nc.vector.tensor_copy(kvhf, kvh)
                                kvl = apool.tile([P, P], BF16, tag="kvl",
                                                 name=f"kvl_{hp}")
                                nc.vector.tensor_tensor(out=kvl, in0=kvd,
                                                        in1=kvhf, op=OP.subtract)
                                py = psum.tile([P, T], F32, tag="mm",
                                               name=f"py_{hp}")
                                nc.tensor.matmul(py, kvh, qTS[:, hp, :],
                                                 start=True, stop=False)
                                nc.tensor.matmul(py, kvl, qTS[:, hp, :],
                                                 start=False, stop=True)
                                nc.vector.tensor_scalar(out=ysT[:, hp, :],
                                                        in0=py,
                                                        scalar1=ATTN_THETA,
                                                        scalar2=None,
                                                        op0=OP.is_ge)
                    # sp12 closed: qS/kS/vS/qTS freed

                    if debug_outputs:
                        dbg_copy(dbg["d_ysT"], ysT, fm=True)

                    # ======== stage 4: proj + LN + spike, residual =========
                    with tc.tile_pool(name="tproj", bufs=1) as tpool, \
                         tc.tile_pool(name="zproj", bufs=4) as zpool:
                        tp_b = (load_bcast("tp", D, tpool)
                                if cfg["tp_c"] is None else None)
                        bp_b = load_bcast("bp", D, tpool) if cfg["has_bp"] else None
                        for tt in range(T // P):
                            zrefs = []
                            for n in range(D // 512):
                                ps = psum.tile([P, 512], F32, tag="mm",
                                               name=f"ps_pr_{tt}_{n}")
                                first = True
                                for wa in (pwh, pwl):
                                    for kk in range(D // P):
                                        nc.tensor.matmul(
                                            ps, ysT[:, kk, ts(tt, P)],
                                            wa[:, kk, ts(n, 512)],
                                            start=first,
                                            stop=(wa is pwl and kk == D // P - 1))
                                        first = False
                                if bp_b is not None:
                                    zc = zpool.tile([P, 512], F32, tag="zproj",
                                                    name=f"zpr_{tt}_{n}")
                                    nc.vector.tensor_tensor(
                                        out=zc, in0=ps,
                                        in1=bp_b[:, ts(n, 512)], op=OP.add)
                                    zrefs.append(zc)
                                else:
                                    zrefs.append(ps)

                            def emit(j, zc, thc, cmp, tt=tt):
                                emit_cmp(y2[:, tt, ts(j, 512)], zc, thc, cmp)
                            ln_spike(zrefs, tp_b, cfg["mode_p"], emit, "proj",
                                     tconst=cfg["tp_c"])
                            nc.vector.tensor_tensor(out=xnew[:, tt, :],
                                                    in0=xt[:, tt, :],
                                                    in1=y2[:, tt, :], op=OP.add)

                    if debug_outputs:
                        dbg_copy(dbg["d_y2"], y2)
                # sp3 closed: ysT, y2 freed
            # xtp closed: xt freed

            # ============ stage 5+6+7 ======================================
            with tc.tile_pool(name="sp6", bufs=1) as sp6:
                m1T = sp6.tile([P, HID // P, T], BF16, tag="m1T")

                with tc.tile_pool(name="sp5", bufs=1) as sp5:
                    xnT_h = sp5.tile([P, D // P, T], BF16, tag="xnT_h")
                    xnT_l = sp5.tile([P, D // P, T], BF16, tag="xnT_l")
                    # ---- stage 5: split xnew + PE transpose ----
                    with tc.tile_pool(name="xsplit", bufs=3) as xsp:
                        for tt in range(T // P):
                            xh = xsp.tile([P, D], BF16, tag="xh", name=f"xh_{tt}")
                            xl = xsp.tile([P, D], BF16, tag="xl", name=f"xl_{tt}")
                            xhf = xsp.tile([P, D], F32, tag="xhf", name=f"xhf_{tt}")
                            nc.vector.tensor_copy(xh, xnew[:, tt, :])
                            nc.vector.tensor_copy(xhf, xh)
                            nc.vector.tensor_tensor(out=xl, in0=xnew[:, tt, :],
                                                    in1=xhf, op=OP.subtract)
                            for fc in range(D // P):
                                for src, dst in ((xh, xnT_h), (xl, xnT_l)):
                                    pt = psum.tile([P, P], BF16, tag="mm",
                                                   name=f"pt_{tt}_{fc}")
                                    nc.tensor.transpose(pt, src[:, ts(fc, P)],
                                                        ident)
                                    nc.vector.tensor_copy(
                                        dst[:, fc, ts(tt, P)], pt)

                    # ---- stage 6: fc1, FEATURE-major so m1 spikes land
                    #      directly in fc2's lhsT layout (no transposes).
                    #      LN stats via fp32r ones-matmul reductions. ----
                    F32R = mybir.dt.float32r
                    NMC = HID // P   # 32 dout chunks
                    with tc.tile_pool(name="z1p", bufs=1) as z1pool, \
                         tc.tile_pool(name="tfc1", bufs=1) as tpool, \
                         tc.tile_pool(name="wf1", bufs=2) as wpool, \
                         tc.tile_pool(name="psred", bufs=1, space="PSUM") as psr, \
                         tc.tile_pool(name="fc1ln", bufs=4) as lp:
                        # t1 / b1 as per-partition [128, 32] (feature-major)
                        t1_fm = None
                        if cfg["t1_c"] is None:
                            t1_fm = tpool.tile([P, NMC], F32, tag="t1_fm")
                            nc.sync.dma_start(
                                t1_fm, thr_names["t1"].ap().rearrange(
                                    "(c p) -> p c", p=P))
                        b1_fm = None
                        if cfg["has_b1"]:
                            b1_fm = tpool.tile([P, NMC], F32, tag="b1_fm")
                            nc.sync.dma_start(
                                b1_fm, bias_names["b1"].ap().rearrange(
                                    "(c p) -> p c", p=P))
                        z1T = z1pool.tile([P, NMC, T], F32, tag="z1T")
                        pr_sum = psr.tile([1, T], F32, tag="pr_sum")
                        pr_sq = psr.tile([1, T], F32, tag="pr_sq")
                        cmp1 = OP.is_ge if cfg["mode_1"] == "pos" else OP.is_le

                        # mean*HID = xn @ rowsum(fc1_w) (+ sum(b1), host-folded)
                        wsh = tpool.tile([P, D // P], BF16, tag="ws1h")
                        wsl = tpool.tile([P, D // P], BF16, tag="ws1l")
                        nc.sync.dma_start(wsh, ws1_hi.ap().rearrange(
                            "(c p) -> p c", p=P))
                        nc.sync.dma_start(wsl, ws1_lo.ap().rearrange(
                            "(c p) -> p c", p=P))
                        # two token halves: half 0's LN-apply overlaps
                        # half 1's matmuls (fc1 weights streamed twice)
                        T2 = T // 2
                        for hf in range(2):
                            hsl = bass.ds(hf * T2, T2)
                            first = True
                            for xa, wa in ((xnT_h, wsh), (xnT_l, wsh),
                                           (xnT_h, wsl)):
                                for kk in range(D // P):
                                    nc.tensor.matmul(
                                        pr_sum[:, hsl], wa[:, kk:kk + 1],
                                        xa[:, kk, hsl],
                                        start=first,
                                        stop=(xa is xnT_h and wa is wsl and
                                              kk == D // P - 1))
                                    first = False

                            for mc in range(NMC):
                                if mc % 4 == 0:
                                    # batched weight load: 4 dout chunks
                                    w4h = wpool.tile([P, D // P, 4 * P], BF16,
                                                     tag="f1h",
                                                     name=f"f1h_{hf}_{mc}")
                                    w4l = wpool.tile([P, D // P, 4 * P], BF16,
                                                     tag="f1l",
                                                     name=f"f1l_{hf}_{mc}")
                                    nc.sync.dma_start(
                                        w4h, wv["f1_hi"][:, :, ts(mc // 4, 4 * P)])
                                    nc.sync.dma_start(
                                        w4l, wv["f1_lo"][:, :, ts(mc // 4, 4 * P)])
                                wh = w4h[:, :, ts(mc % 4, P)]
                                wl = w4l[:, :, ts(mc % 4, P)]
                                ps = psum.tile([P, T2], F32, tag="mm",
                                               name=f"ps_f1_{hf}_{mc}")
                                first = True
                                for xa, wa in ((xnT_h, wh), (xnT_l, wh),
                                               (xnT_h, wl)):
                                    for kk in range(D // P):
                                        nc.tensor.matmul(
                                            ps, wa[:, kk, :], xa[:, kk, hsl],
                                            start=first,
                                            stop=(xa is xnT_h and wa is wl and
                                                  kk == D // P - 1))
                                        first = False
                                if b1_fm is not None:
                                    nc.vector.tensor_scalar(
                                        out=z1T[:, mc, hsl], in0=ps,
                                        scalar1=b1_fm[:, mc:mc + 1],
                                        scalar2=None, op0=OP.add)
                                else:
                                    nc.vector.tensor_copy(z1T[:, mc, hsl], ps)
                                zq = lp.tile([P, T2], F32R, tag="zq",
                                             name=f"zq_{hf}_{mc}")
                                nc.scalar.activation(
                                    out=zq, in_=z1T[:, mc, hsl],
                                    func=mybir.ActivationFunctionType.Square,
                                    bias=0.0, scale=1.0)
                                nc.tensor.matmul(pr_sq[:, hsl], ones_r, zq,
                                                 start=(mc == 0),
                                                 stop=(mc == NMC - 1))

                            # stats for this half
                            mrow = lp.tile([1, T2], F32, tag="mrow",
                                           name=f"mrow_{hf}")
                            nc.vector.tensor_scalar(
                                out=mrow, in0=pr_sum[:, hsl],
                                scalar1=1.0 / HID, scalar2=cfg["b1_sum"] / HID,
                                op0=OP.mult, op1=OP.add)
                            e2row = lp.tile([1, T2], F32, tag="e2row",
                                            name=f"e2row_{hf}")
                            nc.vector.tensor_scalar_mul(e2row, pr_sq[:, hsl],
                                                        1.0 / HID)
                            vrow = lp.tile([1, T2], F32, tag="vrow",
                                           name=f"vrow_{hf}")
                            nc.vector.tensor_tensor(out=vrow, in0=mrow,
                                                    in1=mrow, op=OP.mult)
                            nc.vector.tensor_tensor(out=vrow, in0=e2row,
                                                    in1=vrow, op=OP.subtract)
                            srow = lp.tile([1, T2], F32, tag="srow",
                                           name=f"srow_{hf}")
                            nc.scalar.activation(
                                out=srow, in_=vrow,
                                func=mybir.ActivationFunctionType.Sqrt,
                                bias=eps_t[0:1], scale=1.0)
                            if cfg["t1_c"] is not None:
                                # constant t: thr row = m + t*s, one bcast,
                                # fused 3D compares in mc-quarters
                                trow = lp.tile([1, T2], F32, tag="trow",
                                               name=f"trow_{hf}")
                                nc.vector.tensor_scalar(
                                    out=trow, in0=srow,
                                    scalar1=float(cfg["t1_c"]), scalar2=None,
                                    op0=OP.mult)
                                nc.vector.tensor_tensor(out=trow, in0=trow,
                                                        in1=mrow, op=OP.add)
                                t_b = lp.tile([P, T2], F32, tag="m_b",
                                              name=f"t_b_{hf}")
                                nc.gpsimd.partition_broadcast(t_b, trow)
                                QMC = NMC // 4
                                for qq in range(4):
                                    tb3 = t_b[:, None, :].to_broadcast(
                                        (P, QMC, T2))
                                    nc.vector.tensor_tensor(
                                        out=m1T[:, ts(qq, QMC), hsl],
                                        in0=z1T[:, ts(qq, QMC), hsl],
                                        in1=tb3, op=cmp1)
                            else:
                                m_b = lp.tile([P, T2], F32, tag="m_b",
                                              name=f"m_b_{hf}")
                                s_b = lp.tile([P, T2], F32, tag="s_b",
                                              name=f"s_b_{hf}")
                                nc.gpsimd.partition_broadcast(m_b, mrow)
                                nc.gpsimd.partition_broadcast(s_b, srow)
                                for mc in range(NMC):
                                    thr = thrp.tile([P, T2], F32, tag="thr",
                                                    name=f"th1_{hf}_{mc}")
                                    nc.vector.tensor_scalar(
                                        out=thr, in0=s_b,
                                        scalar1=t1_fm[:, mc:mc + 1],
                                        scalar2=None, op0=OP.mult)
                                    nc.vector.tensor_tensor(out=thr, in0=thr,
                                                            in1=m_b, op=OP.add)
                                    nc.vector.tensor_tensor(
                                        out=m1T[:, mc, hsl],
                                        in0=z1T[:, mc, hsl],
                                        in1=thr, op=cmp1)

                        if debug_outputs:
                            dbg_copy(dbg["d_z1T"], z1T, fm=True)
                # sp5 closed: xnT freed

                if debug_outputs:
                    dbg_copy(dbg["d_m1T"], m1T, fm=True)

                # ---- stage 7: fc2 ----
                with tc.tile_pool(name="z2p", bufs=1) as z2pool, \
                     tc.tile_pool(name="tfc2", bufs=1) as tpool, \
                     tc.tile_pool(name="wf2", bufs=3) as wpool, \
                     tc.tile_pool(name="fc2ln", bufs=3) as lp:
                    t2_b = (load_bcast("t2", D, tpool)
                            if cfg["t2_c"] is None else None)
                    b2_b = load_bcast("b2", D, tpool) if cfg["has_b2"] else None
                    z2 = z2pool.tile([P, T // P, D], F32, tag="z2")
                    st2 = z2pool.tile([P, T // P, D // 512, 6], F32, tag="st_fc2")
                    for n in range(D // 512):
                        pss = []
                        for _pi in range(T // P):
                            pst = psum.tile([P, 512], F32, tag="mm",
                                            name=f"ps2_{n}_{_pi}")
                            pss.append(pst)
                        for kk in range(HID // P):
                            if kk % 4 == 0:
                                w4h = wpool.tile([P, 4, 512], BF16, tag="f2h",
                                                 name=f"f2h_{n}_{kk}")
                                w4l = wpool.tile([P, 4, 512], BF16, tag="f2l",
                                                 name=f"f2l_{n}_{kk}")
                                nc.sync.dma_start(
                                    w4h, wv["f2_hi"][:, bass.ds(kk, 4),
                                                     ts(n, 512)])
                                nc.sync.dma_start(
                                    w4l, wv["f2_lo"][:, bass.ds(kk, 4),
                                                     ts(n, 512)])
                            wh = w4h[:, kk % 4]
                            wl = w4l[:, kk % 4]
                            for tt in range(T // P):
                                nc.tensor.matmul(pss[tt], m1T[:, kk, ts(tt, P)],
                                                 wh, start=(kk == 0), stop=False)
                                nc.tensor.matmul(pss[tt], m1T[:, kk, ts(tt, P)],
                                                 wl, start=False,
                                                 stop=(kk == HID // P - 1))
                        for tt in range(T // P):
                            if b2_b is not None:
                                nc.vector.tensor_tensor(
                                    out=z2[:, tt, ts(n, 512)], in0=pss[tt],
                                    in1=b2_b[:, ts(n, 512)], op=OP.add)
                            else:
                                nc.vector.tensor_copy(z2[:, tt, ts(n, 512)],
                                                      pss[tt])
                            nc.vector.bn_stats(st2[:, tt, n],
                                               z2[:, tt, ts(n, 512)])

                    cmp2 = OP.is_ge if cfg["mode_2"] == "pos" else OP.is_le
                    for tt in range(T // P):
                        mv = lp.tile([P, 2], F32, tag="mv2", name=f"mv2_{tt}")
                        nc.vector.bn_aggr(mv, st2[:, tt])
                        std = lp.tile([P, 1], F32, tag="sd2", name=f"sd2_{tt}")
                        nc.scalar.activation(
                            out=std, in_=mv[:, 1:2],
                            func=mybir.ActivationFunctionType.Sqrt,
                            bias=eps_t, scale=1.0)
                        thrc2 = None
                        if cfg["t2_c"] is not None:
                            thrc2 = lp.tile([P, 1], F32, tag="tc2",
                                            name=f"tc2_{tt}")
                            nc.vector.tensor_scalar(
                                out=thrc2, in0=std, scalar1=float(cfg["t2_c"]),
                                scalar2=mv[:, 0:1], op0=OP.mult, op1=OP.add)
                        for n in range(D // 512):
                            m2c = lp.tile([P, 512], F32, tag="m2c",
                                          name=f"m2c_{tt}_{n}")
                            if thrc2 is not None:
                                nc.vector.tensor_scalar(
                                    out=m2c, in0=z2[:, tt, ts(n, 512)],
                                    scalar1=thrc2, scalar2=None, op0=cmp2)
                            else:
                                thr = thrp.tile([P, 512], F32, tag="thr",
                                                name=f"th2_{tt}_{n}")
                                nc.vector.tensor_scalar(
                                    out=thr, in0=t2_b[:, ts(n, 512)],
                                    scalar1=std, scalar2=mv[:, 0:1],
                                    op0=OP.mult, op1=OP.add)
                                nc.vector.tensor_tensor(
                                    out=m2c, in0=z2[:, tt, ts(n, 512)],
                                    in1=thr, op=cmp2)
                            ot = lp.tile([P, 512], F32, tag="ot",
                                         name=f"ot_{tt}_{n}")
                            nc.vector.tensor_tensor(
                                out=ot, in0=xnew[:, tt, ts(n, 512)],
                                in1=m2c, op=OP.add)
                            nc.sync.dma_start(
                                out_dram.ap().rearrange(
                                    "(c p) f -> p c f", p=P)[:, tt, ts(n, 512)],
                                ot)

    nc.compile()
    return nc


def _sign_mode(g):
    if np.all(g > 0):
        return "pos"
    if np.all(g < 0):
        return "neg"
    raise NotImplementedError("mixed-sign LN gain not supported")


def make_core_inputs(x, q_w, q_g, q_b, k_w, k_g, k_b, v_w, v_g, v_b,
                     proj_w, proj_bias, proj_g, proj_beta,
                     fc1_w, fc1_bias, fc1_g, fc1_beta,
                     fc2_w, fc2_bias, fc2_g, fc2_beta):
    f32 = np.float32
    X = np.asarray(x, f32).reshape(B * L, D)

    wsplit = {}
    for nm, W in (("qw", q_w), ("kw", k_w), ("vw", v_w),
                  ("pw", proj_w), ("f1", fc1_w), ("f2", fc2_w)):
        hi, lo = _split_hi_lo(np.asarray(W, f32))
        wsplit[f"{nm}_hi"] = hi
        wsplit[f"{nm}_lo"] = lo

    def thrvec(g, b):
        return ((THETA - np.asarray(b, np.float64))
                / np.asarray(g, np.float64)).astype(f32)

    thr = {"tq": thrvec(q_g, q_b), "tk": thrvec(k_g, k_b),
           "tv": thrvec(v_g, v_b), "tp": thrvec(proj_g, proj_beta),
           "t1": thrvec(fc1_g, fc1_beta), "t2": thrvec(fc2_g, fc2_beta)}

    ws1 = np.asarray(fc1_w, np.float64).sum(axis=1).astype(f32)
    ws1_hi, ws1_lo = _split_hi_lo(ws1)
    def _const_or_none(v):
        v = np.asarray(v, np.float64)
        return float(v[0]) if np.all(v == v[0]) else None

    cfg = {
        "use_cc": os.environ.get("KERNEL_NO_CC", "0") != "1",
        "tq_c": _const_or_none((THETA - np.asarray(q_b, np.float64)) / np.asarray(q_g, np.float64)),
        "tk_c": _const_or_none((THETA - np.asarray(k_b, np.float64)) / np.asarray(k_g, np.float64)),
        "tv_c": _const_or_none((THETA - np.asarray(v_b, np.float64)) / np.asarray(v_g, np.float64)),
        "tp_c": _const_or_none((THETA - np.asarray(proj_beta, np.float64)) / np.asarray(proj_g, np.float64)),
        "t1_c": _const_or_none((THETA - np.asarray(fc1_beta, np.float64)) / np.asarray(fc1_g, np.float64)),
        "t2_c": _const_or_none((THETA - np.asarray(fc2_beta, np.float64)) / np.asarray(fc2_g, np.float64)),
        "b1_sum": float(np.asarray(fc1_bias, np.float64).sum()),
        "mode_q": _sign_mode(np.asarray(q_g)), "mode_k": _sign_mode(np.asarray(k_g)),
        "mode_v": _sign_mode(np.asarray(v_g)), "mode_p": _sign_mode(np.asarray(proj_g)),
        "mode_1": _sign_mode(np.asarray(fc1_g)), "mode_2": _sign_mode(np.asarray(fc2_g)),
        "has_bp": bool(np.any(np.asarray(proj_bias) != 0)),
        "has_b1": bool(np.any(np.asarray(fc1_bias) != 0)),
        "has_b2": bool(np.any(np.asarray(fc2_bias) != 0)),
    }
    biases = {"bp": np.asarray(proj_bias, f32), "b1": np.asarray(fc1_bias, f32),
              "b2": np.asarray(fc2_bias, f32)}

    use_cc = cfg["use_cc"]
    in_maps = []
    for c in range(NCORES):
        b = c // 2
        h = c % 2
        own = X[b * L + h * T: b * L + (h + 1) * T]
        if use_cc:
            xT = np.ascontiguousarray(own.T)                # [D, T]
        else:
            other = X[b * L + (1 - h) * T: b * L + (2 - h) * T]
            Xp = np.concatenate([own, other], axis=0)       # [TB, D] own-first
            xT = np.ascontiguousarray(Xp.T)                 # [D, TB]
        xT_hi, xT_lo = _split_hi_lo(xT)
        m = {"xT_hi": xT_hi, "xT_lo": xT_lo,
             "x_tok": np.ascontiguousarray(own),
             "ident": np.eye(P, dtype=np.float32).astype(ml_dtypes.bfloat16),
             "ws1_hi": ws1_hi, "ws1_lo": ws1_lo}
        m.update(wsplit)
        m.update(thr)
        for nm in ("bp", "b1", "b2"):
            if cfg[f"has_{nm}"]:
                m[nm] = biases[nm]
        in_maps.append(m)
    return in_maps, cfg


_prog_cache = {}


def kernel(**inputs) -> np.ndarray:
    in_maps, cfg = make_core_inputs(**inputs)
    key = tuple(sorted(cfg.items()))
    if key not in _prog_cache:
        _prog_cache[key] = build_program(cfg)
    nc = _prog_cache[key]

    res = run_bass_kernel_spmd(nc, in_maps, core_ids=list(range(NCORES)))
    last_run_info["exec_time_ns"] = res.exec_time_ns
    last_run_info["mean_exec_time_ns"] = res.mean_exec_time_ns

    out = np.empty((B, L, D), np.float32)
    for c in range(NCORES):
        b = c // 2
        h = c % 2
        out[b, h * T:(h + 1) * T, :] = res.results[c]["out"]
    return out


# revision 50
# speedup vs baseline: 1.3068x; 1.0057x over previous
"""Trainium2 Bass kernel for nn_Block_80041010528755 (spiking transformer block).

Math structure (see reference):
  q = spike(LN(x@q_w) >= 2), k/v likewise (binary {0,1})
  attn has NO softmax -> (q@k^T)@v == q@(k^T@v): per-head 64x64 kv matrix,
  exact in bf16/fp32 because spikes are binary and sums are small integers.
  y2 = spike(LN(yspike@proj_w + pb) >= 2); x' = x + y2
  m1 = spike(LN(x'@fc1_w + b1) >= 2); m2 = spike(LN(m1@fc2_w + b2) >= 2)
  out = x' + m2

Precision: fp32-input matmuls (q/k/v from x, fc1 from x') use 3-product
bf16 hi/lo splits (x_hi@W_hi + x_lo@W_hi + x_hi@W_lo, ~2^-16 rel);
binary-input matmuls (proj, fc2) use 2 products (S@W_hi + S@W_lo).
All accumulate in fp32 PSUM.

Sharding: 8-way token-parallel, 512 tokens/core (half a batch). k/v are
computed over the core's full 1024-token batch (duplicated within the
core pair) so attention needs no collectives.
"""

import os
import sys

for _p in ("/root/.axon_site/_ro/trn_rl_repo", "/opt/trn_rl_repo"):
    if os.path.isdir(_p) and _p not in sys.path:
        sys.path.append(_p)

import numpy as np
import ml_dtypes

import concourse.bass as bass
import concourse.bacc as bacc
import concourse.tile as tile
import concourse.mybir as mybir
from concourse.bass import ts
from concourse.bass_utils import run_bass_kernel_spmd

F32 = mybir.dt.float32
BF16 = mybir.dt.bfloat16
OP = mybir.AluOpType

B, L, D = 4, 1024, 1024
HID = 4096
H, HD = 16, 64
NCORES = 8
T = 512          # own tokens per core
TB = 1024        # batch tokens per core (own + partner half)
P = 128
LN_EPS = 1e-5
THETA = 2.0      # LN-spike threshold: TAU*v_th = 2*1
ATTN_THETA = 1.0  # attn spike: y >= TAU*0.5

# module-global stash for timing info from the last kernel() call
last_run_info = {}


def _split_hi_lo(a32):
    hi = a32.astype(ml_dtypes.bfloat16)
    lo = (a32 - hi.astype(np.float32)).astype(ml_dtypes.bfloat16)
    return np.ascontiguousarray(hi), np.ascontiguousarray(lo)


def _bcast_ap(dram_ap, parts=P):
    """[D] dram tensor viewed as [parts, D] with 0-stride partitions."""
    return bass.AP(tensor=dram_ap.tensor, offset=dram_ap.offset,
                   ap=[[0, parts]] + list(dram_ap.ap))


def build_program(cfg, debug_outputs=False):
    """cfg: dict with has_bias flags + g-sign modes per LN stage."""
    nc = bacc.Bacc("TRN2", target_bir_lowering=False, debug=False)

    # ---- DRAM tensors ----
    TQKD = T if cfg["use_cc"] else TB
    xT_hi = nc.dram_tensor("xT_hi", [D, TQKD], BF16, kind="ExternalInput")
    xT_lo = nc.dram_tensor("xT_lo", [D, TQKD], BF16, kind="ExternalInput")
    x_tok = nc.dram_tensor("x_tok", [T, D], F32, kind="ExternalInput")

    w_names = {}
    for nm, (din, dout) in (("qw", (D, D)), ("kw", (D, D)), ("vw", (D, D)),
                            ("pw", (D, D)), ("f1", (D, HID)), ("f2", (HID, D))):
        for h in ("hi", "lo"):
            w_names[f"{nm}_{h}"] = nc.dram_tensor(
                f"{nm}_{h}", [din, dout], BF16, kind="ExternalInput")

    thr_names = {}
    for nm, dd in (("tq", D), ("tk", D), ("tv", D), ("tp", D),
                   ("t1", HID), ("t2", D)):
        thr_names[nm] = nc.dram_tensor(nm, [dd], F32, kind="ExternalInput")

    ident_in = nc.dram_tensor("ident", [P, P], BF16, kind="ExternalInput")
    ws1_hi = nc.dram_tensor("ws1_hi", [D], BF16, kind="ExternalInput")
    ws1_lo = nc.dram_tensor("ws1_lo", [D], BF16, kind="ExternalInput")

    bias_names = {}
    for nm, dd in (("bp", D), ("b1", HID), ("b2", D)):
        if cfg[f"has_{nm}"]:
            bias_names[nm] = nc.dram_tensor(nm, [dd], F32, kind="ExternalInput")

    out_dram = nc.dram_tensor("out", [T, D], F32, kind="ExternalOutput")

    dbg = {}
    if debug_outputs:
        TKV = T if cfg["use_cc"] else TB
        for nm, shp, dt in (("d_qsT", [D, T], BF16), ("d_ks", [TKV, D], BF16),
                            ("d_vs", [TKV, D], BF16), ("d_ysT", [D, T], BF16),
                            ("d_y2", [T, D], BF16), ("d_m1T", [HID, T], BF16),
                            ("d_z1T", [HID, T], F32)):
            dbg[nm] = nc.dram_tensor(nm, shp, dt, kind="ExternalOutput")

    # weight dram views [p, kc, dout]
    wv = {k: v.ap().rearrange("(kc p) f -> p kc f", p=P)
          for k, v in w_names.items()}

    def dbg_copy(dram, sb, fm=False):
        pat = "(c p) t -> p c t" if fm else "(c p) f -> p c f"
        dv = dram.ap().rearrange(pat, p=P)
        for c in range(sb.shape[1]):
            nc.sync.dma_start(dv[:, c, :], sb[:, c, :])

    with tile.TileContext(nc) as tc:
        with tc.tile_pool(name="psum", bufs=6, space="PSUM") as psum, \
             tc.tile_pool(name="stats", bufs=6) as stats, \
             tc.tile_pool(name="thrp", bufs=3) as thrp, \
             tc.tile_pool(name="consts", bufs=1) as consts, \
             tc.tile_pool(name="resid", bufs=1) as resid:

            eps_t = consts.tile([P, 1], F32)
            nc.vector.memset(eps_t, LN_EPS)
            ident = consts.tile([P, P], BF16, tag="ident")
            nc.sync.dma_start(ident, ident_in.ap())
            ones0 = consts.tile([P, 1], F32, tag="ones0")
            nc.vector.memset(ones0, 1.0)
            ones_r = consts.tile([P, 1], mybir.dt.float32r, tag="ones_r")
            nc.vector.tensor_copy(ones_r, ones0)

            # round-robin 128x128 DMA transposes over the two HWDGE queues
            _tp_state = [0]

            def tpose(dst, src):
                eng = (nc.sync, nc.sync)[_tp_state[0] % 2]
                _tp_state[0] += 1
                eng.dma_start(out=dst, in_=src, transpose=True)

            def load_bcast(name, dd, pool):
                t = pool.tile([P, dd], F32, tag=f"bc_{name}", name=f"bc_{name}")
                nc.gpsimd.dma_start(t, _bcast_ap(thr_names[name].ap()
                                                 if name in thr_names
                                                 else bias_names[name].ap()))
                return t

            # ---------- LN + spike helper (token-major) ----------
            def ln_spike(z_chunks, thr_b, mode, out_fn, stat_tag, tconst=None):
                """z_chunks: APs [128, 512] covering the feature dim for one
                token block. thr_b: [128, d_total] bcast of (theta-b)/g
                (unused when tconst given). Emits spike = cmp(z, mean+t*std).
                With constant t, the threshold is a per-token scalar
                thr = mean + t*std -> single tensor_scalar per chunk."""
                nchunks = len(z_chunks)
                st = stats.tile([P, nchunks, 6], F32, tag=f"st_{stat_tag}",
                                name=f"st_{stat_tag}")
                for j, zc in enumerate(z_chunks):
                    nc.vector.bn_stats(st[:, j], zc)
                mv = stats.tile([P, 2], F32, tag=f"mv_{stat_tag}",
                                name=f"mv_{stat_tag}")
                nc.vector.bn_aggr(mv, st)
                std = stats.tile([P, 1], F32, tag=f"sd_{stat_tag}",
                                 name=f"sd_{stat_tag}")
                nc.scalar.activation(out=std, in_=mv[:, 1:2],
                                     func=mybir.ActivationFunctionType.Sqrt,
                                     bias=eps_t, scale=1.0)
                cmp = OP.is_ge if mode == "pos" else OP.is_le
                if tconst is not None:
                    thrc = stats.tile([P, 1], F32, tag=f"tc_{stat_tag}",
                                      name=f"tc_{stat_tag}")
                    nc.vector.tensor_scalar(out=thrc, in0=std,
                                            scalar1=float(tconst),
                                            scalar2=mv[:, 0:1],
                                            op0=OP.mult, op1=OP.add)
                    for j, zc in enumerate(z_chunks):
                        out_fn(j, zc, thrc, cmp)
                    return
                for j, zc in enumerate(z_chunks):
                    thr = thrp.tile([P, 512], F32, tag="thr", name=f"th_{stat_tag}_{j}")
                    nc.vector.tensor_scalar(out=thr, in0=thr_b[:, ts(j, 512)],
                                            scalar1=std, scalar2=mv[:, 0:1],
                                            op0=OP.mult, op1=OP.add)
                    out_fn(j, zc, thr, cmp)

            def emit_cmp(out_ap, zc, thc, cmp):
                if thc.free_size() == 1:
                    nc.vector.tensor_scalar(out=out_ap, in0=zc, scalar1=thc,
                                            scalar2=None, op0=cmp)
                else:
                    nc.vector.tensor_tensor(out=out_ap, in0=zc, in1=thc, op=cmp)

            xnew = resid.tile([P, T // P, D], F32, tag="xnew")

            with tc.tile_pool(name="xtp", bufs=1) as xtp:
                xt = xtp.tile([P, T // P, D], F32, tag="xt")
                # off the sync queue: only needed at proj time
                nc.gpsimd.dma_start(xt, x_tok.ap().rearrange("(c p) f -> p c f",
                                                             p=P))

                with tc.tile_pool(name="sp3", bufs=1) as sp3, \
                     tc.tile_pool(name="wp", bufs=1) as wpool:
                    ysT = sp3.tile([P, D // P, T], BF16, tag="ysT")
                    y2 = sp3.tile([P, T // P, D], BF16, tag="y2")
                    # proj weights: prefetched during stages 1-3
                    pwh = wpool.tile([P, D // P, D], BF16, tag="w_pw_hi")
                    pwl = wpool.tile([P, D // P, D], BF16, tag="w_pw_lo")

                    with tc.tile_pool(name="sp12", bufs=1) as sp12:
                        TQK = T if cfg["use_cc"] else TB   # k/v token span
                        NTKV = TQK // P
                        kS = sp12.tile([P, NTKV, D], BF16, tag="kS")
                        vS = sp12.tile([P, NTKV, D], BF16, tag="vS")
                        qTS = sp12.tile([P, D // P, T], BF16, tag="qTS")
                        kvred = sp12.tile([P, D // P, P], F32, tag="kvred")

                        # ======== stage 1+2: k, v, q + kv collective =======
                        with tc.tile_pool(name="xTp", bufs=1) as xTpool, \
                             tc.tile_pool(name="tqkv", bufs=1) as tpool, \
                             tc.tile_pool(name="qsc", bufs=3) as qscp, \
                             tc.tile_pool(name="ccdram", bufs=1,
                                          space="DRAM") as ccd, \
                             tc.tile_pool(name="wqkvh", bufs=(2 if cfg["use_cc"] else 1)) as wqkvh, \
                             tc.tile_pool(name="wqkvl", bufs=1) as wqkvl:
                            xTh = xTpool.tile([P, D // P, TQK], BF16, tag="xTh")
                            xTl = xTpool.tile([P, D // P, TQK], BF16, tag="xTl")
                            xThd = xT_hi.ap().rearrange("(c p) t -> p c t", p=P)
                            xTld = xT_lo.ap().rearrange("(c p) t -> p c t", p=P)
                            nc.sync.dma_start(xTh[:, :, 0:T], xThd[:, :, 0:T])
                            nc.sync.dma_start(xTl[:, :, 0:T], xTld[:, :, 0:T])
                            tq_b = (load_bcast("tq", D, tpool)
                                    if cfg["tq_c"] is None else None)
                            tk_b = (load_bcast("tk", D, tpool)
                                    if cfg["tk_c"] is None else None)
                            tv_b = (load_bcast("tv", D, tpool)
                                    if cfg["tv_c"] is None else None)

                            if cfg["use_cc"]:
                                order = (("kw", kS, tk_b, NTKV, cfg["mode_k"]),
                                         ("vw", vS, tv_b, NTKV, cfg["mode_v"]),
                                         ("qw", None, tq_b, T // P,
                                          cfg["mode_q"]))
                            else:
                                order = (("qw", None, tq_b, T // P,
                                          cfg["mode_q"]),
                                         ("kw", kS, tk_b, NTKV, cfg["mode_k"]),
                                         ("vw", vS, tv_b, NTKV, cfg["mode_v"]))

                            for nm, spk, thr_b, ntt, mode in order:
                                if True:
                                    whi = wqkvh.tile([P, D // P, D], BF16,
                                                     tag="wqkv_hi",
                                                     name=f"{nm}_hi_t")
                                    wlo = wqkvl.tile([P, D // P, D], BF16,
                                                     tag="wqkv_lo",
                                                     name=f"{nm}_lo_t")
                                    if nm == ("kw" if cfg["use_cc"] else "qw"):
                                        for c4 in range(4):
                                            nc.sync.dma_start(
                                                whi[:, ts(c4, 2)],
                                                wv[f"{nm}_hi"][:, ts(c4, 2)])
                                            nc.sync.dma_start(
                                                wlo[:, ts(c4, 2)],
                                                wv[f"{nm}_lo"][:, ts(c4, 2)])
                                    else:
                                        nc.sync.dma_start(whi, wv[f"{nm}_hi"])
                                        nc.sync.dma_start(wlo, wv[f"{nm}_lo"])
                                    if nm == "qw":
                                        nc.sync.dma_start(pwh, wv["pw_hi"])
                                        nc.sync.dma_start(pwl, wv["pw_lo"])
                                    if not cfg["use_cc"] and nm == "qw":
                                        # partner xT halves: needed from k on
                                        nc.sync.dma_start(xTh[:, :, T:TB],
                                                          xThd[:, :, T:TB])
                                        nc.sync.dma_start(xTl[:, :, T:TB],
                                                          xTld[:, :, T:TB])
                                    for tt in range(ntt):
                                        pss = []
                                        for n in range(D // 512):
                                            ps = psum.tile([P, 512], F32, tag="mm",
                                                           name=f"ps_{nm}_{tt}_{n}")
                                            first = True
                                            for xa, wa in ((xTh, whi), (xTl, whi),
                                                           (xTh, wlo)):
                                                for kk in range(D // P):
                                                    nc.tensor.matmul(
                                                        ps, xa[:, kk, ts(tt, P)],
                                                        wa[:, kk, ts(n, 512)],
                                                        start=first,
                                                        stop=(xa is xTh and
                                                              wa is wlo and
                                                              kk == D // P - 1))
                                                    first = False
                                            pss.append(ps)

                                        if spk is None:
                                            # q: emit to scratch, PE-transpose
                                            # to feature-major immediately
                                            def emit(j, zc, thc, cmp, tt=tt):
                                                qc = qscp.tile(
                                                    [P, 512], BF16, tag="qc",
                                                    name=f"qc_{tt}_{j}")
                                                emit_cmp(qc, zc, thc, cmp)
                                                for j2 in range(4):
                                                    fcx = j * 4 + j2
                                                    pt = psum.tile(
                                                        [P, P], BF16, tag="mm",
                                                        name=f"qpt_{tt}_{fcx}")
                                                    nc.tensor.transpose(
                                                        pt, qc[:, ts(j2, P)],
                                                        ident)
                                                    nc.vector.tensor_copy(
                                                        qTS[:, fcx, ts(tt, P)],
                                                        pt)
                                        else:
                                            def emit(j, zc, thc, cmp, spk=spk,
                                                     tt=tt):
                                                emit_cmp(spk[:, tt, ts(j, 512)],
                                                         zc, thc, cmp)
                                        ln_spike(pss, thr_b, mode, emit, "qkv",
                                                 tconst=cfg[f"t{nm[0]}_c"])

                                if cfg["use_cc"] and nm == "vw":
                                    # kv partials + pairwise all-reduce;
                                    # latency hides under the q stage
                                    kvall = xTpool.tile([P, D // P, P], F32,
                                                        tag="kvall")
                                    for hp in range(D // P):
                                        pkv = psum.tile([P, P], F32, tag="mm",
                                                        name=f"pkv_{hp}")
                                        for tt in range(NTKV):
                                            nc.tensor.matmul(
                                                pkv, kS[:, tt, ts(hp, P)],
                                                vS[:, tt, ts(hp, P)],
                                                start=(tt == 0),
                                                stop=(tt == NTKV - 1))
                                        nc.vector.tensor_copy(kvall[:, hp], pkv)
                                    cc_in = ccd.tile([P, D], F32, tag="cc_in")
                                    cc_out = ccd.tile([P, D], F32, tag="cc_out")
                                    nc.gpsimd.dma_start(
                                        cc_in, kvall.rearrange("p c q -> p (c q)"))
                                    pair = [[2 * i, 2 * i + 1]
                                            for i in range(NCORES // 2)]
                                    nc.gpsimd.collective_compute(
                                        "AllReduce", OP.add,
                                        replica_groups=pair,
                                        ins=[cc_in.opt()], outs=[cc_out.opt()])
                                    nc.gpsimd.dma_start(
                                        kvred.rearrange("p c q -> p (c q)"),
                                        cc_out)

                            if not cfg["use_cc"]:
                                # kv straight from local psums
                                for hp in range(D // P):
                                    pkv = psum.tile([P, P], F32, tag="mm",
                                                    name=f"pkv_{hp}")
                                    for tt in range(NTKV):
                                        nc.tensor.matmul(
                                            pkv, kS[:, tt, ts(hp, P)],
                                            vS[:, tt, ts(hp, P)],
                                            start=(tt == 0),
                                            stop=(tt == NTKV - 1))
                                    nc.vector.tensor_copy(kvred[:, hp], pkv)

                        if debug_outputs:
                            dbg_copy(dbg["d_qsT"], qTS, fm=True)
                            dbg_copy(dbg["d_ks"], kS)
                            dbg_copy(dbg["d_vs"], vS)

                        # ======== stage 3: y + attn spike ==================
                        with tc.tile_pool(name="attn", bufs=4) as apool:
                            for hp in range(D // P):   # 8 head pairs
                                kvd = apool.tile([P, P], F32, tag="kvd",
                                                 name=f"kvd_{hp}")
                                nc.vector.memset(kvd, 0.0)
                                nc.vector.tensor_scalar_mul(
                                    kvd[0:HD, 0:HD], kvred[0:HD, hp, 0:HD],
                                    0.125)
                                nc.vector.tensor_scalar_mul(
                                    kvd[HD:P, HD:P], kvred[HD:P, hp, HD:P],
                                    0.125)
                                kvh = apool.tile([P, P], BF16, tag="kvh",
                                                 name=f"kvh_{hp}")
                                nc.vector.tensor_copy(kvh, kvd)
                                kvhf = apool.tile([P, P], F32, tag="kvhf",
                                                  name=f"kvhf_{hp}")
                                nc.vector.tensor_copy(kvhf, kvh)
                                kvl = apool.tile([P, P], BF16, tag="kvl",
                                                 name=f"kvl_{hp}")
                                nc.vector.tensor_tensor(out=kvl, in0=kvd,
                                                        in1=kvhf, op=OP.subtract)
                                py = psum.tile([P, T], F32, tag="mm",
                                               name=f"py_{hp}")
                                nc.tensor.matmul(py, kvh, qTS[:, hp, :],
                                                 start=True, stop=False)
                                nc.tensor.matmul(py, kvl, qTS[:, hp, :],
                                                 start=False, stop=True)
                                nc.vector.tensor_scalar(out=ysT[:, hp, :],
                                                        in0=py,
                                                        scalar1=ATTN_THETA,
                                                        scalar2=None,
                                                        op0=OP.is_ge)
                    # sp12 closed: qS/kS/vS/qTS freed

                    if debug_outputs:
                        dbg_copy(dbg["d_ysT"], ysT, fm=True)

                    # ======== stage 4: proj + LN + spike, residual =========
                    with tc.tile_pool(name="tproj", bufs=1) as tpool, \
                         tc.tile_pool(name="zproj", bufs=4) as zpool:
                        tp_b = (load_bcast("tp", D, tpool)
                                if cfg["tp_c"] is None else None)
                        bp_b = load_bcast("bp", D, tpool) if cfg["has_bp"] else None
                        for tt in range(T // P):
                            zrefs = []
                            for n in range(D // 512):
                                ps = psum.tile([P, 512], F32, tag="mm",
                                               name=f"ps_pr_{tt}_{n}")
                                first = True
                                for wa in (pwh, pwl):
                                    for kk in range(D // P):
                                        nc.tensor.matmul(
                                            ps, ysT[:, kk, ts(tt, P)],
                                            wa[:, kk, ts(n, 512)],
                                            start=first,
                                            stop=(wa is pwl and kk == D // P - 1))
                                        first = False
                                if bp_b is not None:
                                    zc = zpool.tile([P, 512], F32, tag="zproj",
                                                    name=f"zpr_{tt}_{n}")
                                    nc.vector.tensor_tensor(
                                        out=zc, in0=ps,
                                        in1=bp_b[:, ts(n, 512)], op=OP.add)
                                    zrefs.append(zc)
                                else:
                                    zrefs.append(ps)

                            def emit(j, zc, thc, cmp, tt=tt):
                                emit_cmp(y2[:, tt, ts(j, 512)], zc, thc, cmp)
                            ln_spike(zrefs, tp_b, cfg["mode_p"], emit, "proj",
                                     tconst=cfg["tp_c"])
                            nc.vector.tensor_tensor(out=xnew[:, tt, :],
                                                    in0=xt[:, tt, :],
                                                    in1=y2[:, tt, :], op=OP.add)

                    if debug_outputs:
                        dbg_copy(dbg["d_y2"], y2)
                # sp3 closed: ysT, y2 freed
            # xtp closed: xt freed

            # ============ stage 5+6+7 ======================================
            with tc.tile_pool(name="sp6", bufs=1) as sp6:
                m1T = sp6.tile([P, HID // P, T], BF16, tag="m1T")

                with tc.tile_pool(name="sp5", bufs=1) as sp5:
                    xnT_h = sp5.tile([P, D // P, T], BF16, tag="xnT_h")
                    xnT_l = sp5.tile([P, D // P, T], BF16, tag="xnT_l")
                    # ---- stage 5: split xnew + PE transpose ----
                    with tc.tile_pool(name="xsplit", bufs=3) as xsp:
                        for tt in range(T // P):
                            xh = xsp.tile([P, D], BF16, tag="xh", name=f"xh_{tt}")
                            xl = xsp.tile([P, D], BF16, tag="xl", name=f"xl_{tt}")
                            xhf = xsp.tile([P, D], F32, tag="xhf", name=f"xhf_{tt}")
                            nc.vector.tensor_copy(xh, xnew[:, tt, :])
                            nc.vector.tensor_copy(xhf, xh)
                            nc.vector.tensor_tensor(out=xl, in0=xnew[:, tt, :],
                                                    in1=xhf, op=OP.subtract)
                            for fc in range(D // P):
                                for src, dst in ((xh, xnT_h), (xl, xnT_l)):
                                    pt = psum.tile([P, P], BF16, tag="mm",
                                                   name=f"pt_{tt}_{fc}")
                                    nc.tensor.transpose(pt, src[:, ts(fc, P)],
                                                        ident)
                                    nc.vector.tensor_copy(
                                        dst[:, fc, ts(tt, P)], pt)

                    # ---- stage 6: fc1, FEATURE-major so m1 spikes land
                    #      directly in fc2's lhsT layout (no transposes).
                    #      LN stats via fp32r ones-matmul reductions. ----
                    F32R = mybir.dt.float32r
                    NMC = HID // P   # 32 dout chunks
                    with tc.tile_pool(name="z1p", bufs=1) as z1pool, \
                         tc.tile_pool(name="tfc1", bufs=1) as tpool, \
                         tc.tile_pool(name="wf1", bufs=2) as wpool, \
                         tc.tile_pool(name="psred", bufs=1, space="PSUM") as psr, \
                         tc.tile_pool(name="fc1ln", bufs=2) as lp:
                        # t1 / b1 as per-partition [128, 32] (feature-major)
                        t1_fm = None
                        if cfg["t1_c"] is None:
                            t1_fm = tpool.tile([P, NMC], F32, tag="t1_fm")
                            nc.sync.dma_start(
                                t1_fm, thr_names["t1"].ap().rearrange(
                                    "(c p) -> p c", p=P))
                        b1_fm = None
                        if cfg["has_b1"]:
                            b1_fm = tpool.tile([P, NMC], F32, tag="b1_fm")
                            nc.sync.dma_start(
                                b1_fm, bias_names["b1"].ap().rearrange(
                                    "(c p) -> p c", p=P))
                        z1T = z1pool.tile([P, NMC, T], F32, tag="z1T")
                        pr_sum = psr.tile([1, T], F32, tag="pr_sum")
                        pr_sq = psr.tile([1, T], F32, tag="pr_sq")
                        cmp1 = OP.is_ge if cfg["mode_1"] == "pos" else OP.is_le

                        # mean*HID = xn @ rowsum(fc1_w) (+ sum(b1), host-folded)
                        wsh = tpool.tile([P, D // P], BF16, tag="ws1h")
                        wsl = tpool.tile([P, D // P], BF16, tag="ws1l")
                        nc.sync.dma_start(wsh, ws1_hi.ap().rearrange(
                            "(c p) -> p c", p=P))
                        nc.sync.dma_start(wsl, ws1_lo.ap().rearrange(
                            "(c p) -> p c", p=P))
                        # single full-width pass; the const-threshold fused
                        # compare keeps the LN-apply short
                        T2 = T
                        for hf in range(1):
                            hsl = bass.ds(hf * T2, T2)
                            first = True
                            for xa, wa in ((xnT_h, wsh), (xnT_l, wsh),
                                           (xnT_h, wsl)):
                                for kk in range(D // P):
                                    nc.tensor.matmul(
                                        pr_sum[:, hsl], wa[:, kk:kk + 1],
                                        xa[:, kk, hsl],
                                        start=first,
                                        stop=(xa is xnT_h and wa is wsl and
                                              kk == D // P - 1))
                                    first = False

                            for mc in range(NMC):
                                if mc % 4 == 0:
                                    # batched weight load: 4 dout chunks
                                    w4h = wpool.tile([P, D // P, 4 * P], BF16,
                                                     tag="f1h",
                                                     name=f"f1h_{hf}_{mc}")
                                    w4l = wpool.tile([P, D // P, 4 * P], BF16,
                                                     tag="f1l",
                                                     name=f"f1l_{hf}_{mc}")
                                    nc.sync.dma_start(
                                        w4h, wv["f1_hi"][:, :, ts(mc // 4, 4 * P)])
                                    nc.sync.dma_start(
                                        w4l, wv["f1_lo"][:, :, ts(mc // 4, 4 * P)])
                                wh = w4h[:, :, ts(mc % 4, P)]
                                wl = w4l[:, :, ts(mc % 4, P)]
                                ps = psum.tile([P, T2], F32, tag="mm",
                                               name=f"ps_f1_{hf}_{mc}")
                                first = True
                                for xa, wa in ((xnT_h, wh), (xnT_l, wh),
                                               (xnT_h, wl)):
                                    for kk in range(D // P):
                                        nc.tensor.matmul(
                                            ps, wa[:, kk, :], xa[:, kk, hsl],
                                            start=first,
                                            stop=(xa is xnT_h and wa is wl and
                                                  kk == D // P - 1))
                                        first = False
                                if b1_fm is not None:
                                    nc.vector.tensor_scalar(
                                        out=z1T[:, mc, hsl], in0=ps,
                                        scalar1=b1_fm[:, mc:mc + 1],
                                        scalar2=None, op0=OP.add)
                                else:
                                    nc.vector.tensor_copy(z1T[:, mc, hsl], ps)
                                zq = lp.tile([P, T2], F32R, tag="zq",
                                             name=f"zq_{hf}_{mc}")
                                nc.scalar.activation(
                                    out=zq, in_=z1T[:, mc, hsl],
                                    func=mybir.ActivationFunctionType.Square,
                                    bias=0.0, scale=1.0)
                                nc.tensor.matmul(pr_sq[:, hsl], ones_r, zq,
                                                 start=(mc == 0),
                                                 stop=(mc == NMC - 1))

                            # stats for this half
                            mrow = lp.tile([1, T2], F32, tag="mrow",
                                           name=f"mrow_{hf}")
                            nc.vector.tensor_scalar(
                                out=mrow, in0=pr_sum[:, hsl],
                                scalar1=1.0 / HID, scalar2=cfg["b1_sum"] / HID,
                                op0=OP.mult, op1=OP.add)
                            e2row = lp.tile([1, T2], F32, tag="e2row",
                                            name=f"e2row_{hf}")
                            nc.vector.tensor_scalar_mul(e2row, pr_sq[:, hsl],
                                                        1.0 / HID)
                            vrow = lp.tile([1, T2], F32, tag="vrow",
                                           name=f"vrow_{hf}")
                            nc.vector.tensor_tensor(out=vrow, in0=mrow,
                                                    in1=mrow, op=OP.mult)
                            nc.vector.tensor_tensor(out=vrow, in0=e2row,
                                                    in1=vrow, op=OP.subtract)
                            srow = lp.tile([1, T2], F32, tag="srow",
                                           name=f"srow_{hf}")
                            nc.scalar.activation(
                                out=srow, in_=vrow,
                                func=mybir.ActivationFunctionType.Sqrt,
                                bias=eps_t[0:1], scale=1.0)
                            if cfg["t1_c"] is not None:
                                # constant t: thr row = m + t*s, one bcast,
                                # fused 3D compares in mc-quarters
                                trow = lp.tile([1, T2], F32, tag="trow",
                                               name=f"trow_{hf}")
                                nc.vector.tensor_scalar(
                                    out=trow, in0=srow,
                                    scalar1=float(cfg["t1_c"]), scalar2=None,
                                    op0=OP.mult)
                                nc.vector.tensor_tensor(out=trow, in0=trow,
                                                        in1=mrow, op=OP.add)
                                t_b = lp.tile([P, T2], F32, tag="m_b",
                                              name=f"t_b_{hf}")
                                nc.gpsimd.partition_broadcast(t_b, trow)
                                QMC = NMC // 4
                                for qq in range(4):
                                    tb3 = t_b[:, None, :].to_broadcast(
                                        (P, QMC, T2))
                                    nc.vector.tensor_tensor(
                                        out=m1T[:, ts(qq, QMC), hsl],
                                        in0=z1T[:, ts(qq, QMC), hsl],
                                        in1=tb3, op=cmp1)
                            else:
                                m_b = lp.tile([P, T2], F32, tag="m_b",
                                              name=f"m_b_{hf}")
                                s_b = lp.tile([P, T2], F32, tag="s_b",
                                              name=f"s_b_{hf}")
                                nc.gpsimd.partition_broadcast(m_b, mrow)
                                nc.gpsimd.partition_broadcast(s_b, srow)
                                for mc in range(NMC):
                                    thr = thrp.tile([P, T2], F32, tag="thr",
                                                    name=f"th1_{hf}_{mc}")
                                    nc.vector.tensor_scalar(
                                        out=thr, in0=s_b,
                                        scalar1=t1_fm[:, mc:mc + 1],
                                        scalar2=None, op0=OP.mult)
                                    nc.vector.tensor_tensor(out=thr, in0=thr,
                                                            in1=m_b, op=OP.add)
                                    nc.vector.tensor_tensor(
                                        out=m1T[:, mc, hsl],
                                        in0=z1T[:, mc, hsl],
                                        in1=thr, op=cmp1)

                        if debug_outputs:
                            dbg_copy(dbg["d_z1T"], z1T, fm=True)
                # sp5 closed: xnT freed

                if debug_outputs:
                    dbg_copy(dbg["d_m1T"], m1T, fm=True)

                # ---- stage 7: fc2 ----
                with tc.tile_pool(name="z2p", bufs=1) as z2pool, \
                     tc.tile_pool(name="tfc2", bufs=1) as tpool, \
                     tc.tile_pool(name="wf2", bufs=3) as wpool, \
                     tc.tile_pool(name="fc2ln", bufs=3) as lp:
                    t2_b = (load_bcast("t2", D, tpool)
                            if cfg["t2_c"] is None else None)
                    b2_b = load_bcast("b2", D, tpool) if cfg["has_b2"] else None
                    z2 = z2pool.tile([P, T // P, D], F32, tag="z2")
                    st2 = z2pool.tile([P, T // P, D // 512, 6], F32, tag="st_fc2")
                    for n in range(D // 512):
                        pss = []
                        for _pi in range(T // P):
                            pst = psum.tile([P, 512], F32, tag="mm",
                                            name=f"ps2_{n}_{_pi}")
                            pss.append(pst)
                        for kk in range(HID // P):
                            if kk % 4 == 0:
                                w4h = wpool.tile([P, 4, 512], BF16, tag="f2h",
                                                 name=f"f2h_{n}_{kk}")
                                w4l = wpool.tile([P, 4, 512], BF16, tag="f2l",
                                                 name=f"f2l_{n}_{kk}")
                                nc.sync.dma_start(
                                    w4h, wv["f2_hi"][:, bass.ds(kk, 4),
                                                     ts(n, 512)])
                                nc.sync.dma_start(
                                    w4l, wv["f2_lo"][:, bass.ds(kk, 4),
                                                     ts(n, 512)])
                            wh = w4h[:, kk % 4]
                            wl = w4l[:, kk % 4]
                            for tt in range(T // P):
                                nc.tensor.matmul(pss[tt], m1T[:, kk, ts(tt, P)],
                                                 wh, start=(kk == 0), stop=False)
                                nc.tensor.matmul(pss[tt], m1T[:, kk, ts(tt, P)],
                                                 wl, start=False,
                                                 stop=(kk == HID // P - 1))
                        for tt in range(T // P):
                            if b2_b is not None:
                                nc.vector.tensor_tensor(
                                    out=z2[:, tt, ts(n, 512)], in0=pss[tt],
                                    in1=b2_b[:, ts(n, 512)], op=OP.add)
                            else:
                                nc.vector.tensor_copy(z2[:, tt, ts(n, 512)],
                                                      pss[tt])
                            nc.vector.bn_stats(st2[:, tt, n],
                                               z2[:, tt, ts(n, 512)])

                    cmp2 = OP.is_ge if cfg["mode_2"] == "pos" else OP.is_le
                    for tt in range(T // P):
                        mv = lp.tile([P, 2], F32, tag="mv2", name=f"mv2_{tt}")
                        nc.vector.bn_aggr(mv, st2[:, tt])
                        std = lp.tile([P, 1], F32, tag="sd2", name=f"sd2_{tt}")
                        nc.scalar.activation(
                            out=std, in_=mv[:, 1:2],
                            func=mybir.ActivationFunctionType.Sqrt,
                            bias=eps_t, scale=1.0)
                        thrc2 = None
                        if cfg["t2_c"] is not None:
                            thrc2 = lp.tile([P, 1], F32, tag="tc2",
                                            name=f"tc2_{tt}")
                            nc.vector.tensor_scalar(
                                out=thrc2, in0=std, scalar1=float(cfg["t2_c"]),
                                scalar2=mv[:, 0:1], op0=OP.mult, op1=OP.add)
                        for n in range(D // 512):
                            m2c = lp.tile([P, 512], F32, tag="m2c",
                                          name=f"m2c_{tt}_{n}")
                            if thrc2 is not None:
                                nc.vector.tensor_scalar(
                                    out=m2c, in0=z2[:, tt, ts(n, 512)],
                                    scalar1=thrc2, scalar2=None, op0=cmp2)
                            else:
                                thr = thrp.tile([P, 512], F32, tag="thr",
                                                name=f"th2_{tt}_{n}")
                                nc.vector.tensor_scalar(
                                    out=thr, in0=t2_b[:, ts(n, 512)],
                                    scalar1=std, scalar2=mv[:, 0:1],
                                    op0=OP.mult, op1=OP.add)
                                nc.vector.tensor_tensor(
                                    out=m2c, in0=z2[:, tt, ts(n, 512)],
                                    in1=thr, op=cmp2)
                            ot = lp.tile([P, 512], F32, tag="ot",
                                         name=f"ot_{tt}_{n}")
                            nc.vector.tensor_tensor(
                                out=ot, in0=xnew[:, tt, ts(n, 512)],
                                in1=m2c, op=OP.add)
                            nc.sync.dma_start(
                                out_dram.ap().rearrange(
                                    "(c p) f -> p c f", p=P)[:, tt, ts(n, 512)],
                                ot)

    nc.compile()
    return nc


def _sign_mode(g):
    if np.all(g > 0):
        return "pos"
    if np.all(g < 0):
        return "neg"
    raise NotImplementedError("mixed-sign LN gain not supported")


def make_core_inputs(x, q_w, q_g, q_b, k_w, k_g, k_b, v_w, v_g, v_b,
                     proj_w, proj_bias, proj_g, proj_beta,
                     fc1_w, fc1_bias, fc1_g, fc1_beta,
                     fc2_w, fc2_bias, fc2_g, fc2_beta):
    f32 = np.float32
    X = np.asarray(x, f32).reshape(B * L, D)

    wsplit = {}
    for nm, W in (("qw", q_w), ("kw", k_w), ("vw", v_w),
                  ("pw", proj_w), ("f1", fc1_w), ("f2", fc2_w)):
        hi, lo = _split_hi_lo(np.asarray(W, f32))
        wsplit[f"{nm}_hi"] = hi
        wsplit[f"{nm}_lo"] = lo

    def thrvec(g, b):
        return ((THETA - np.asarray(b, np.float64))
                / np.asarray(g, np.float64)).astype(f32)

    thr = {"tq": thrvec(q_g, q_b), "tk": thrvec(k_g, k_b),
           "tv": thrvec(v_g, v_b), "tp": thrvec(proj_g, proj_beta),
           "t1": thrvec(fc1_g, fc1_beta), "t2": thrvec(fc2_g, fc2_beta)}

    ws1 = np.asarray(fc1_w, np.float64).sum(axis=1).astype(f32)
    ws1_hi, ws1_lo = _split_hi_lo(ws1)
    def _const_or_none(v):
        v = np.asarray(v, np.float64)
        return float(v[0]) if np.all(v == v[0]) else None

    cfg = {
        "use_cc": os.environ.get("KERNEL_NO_CC", "0") != "1",
        "tq_c": _const_or_none((THETA - np.asarray(q_b, np.float64)) / np.asarray(q_g, np.float64)),
        "tk_c": _const_or_none((THETA - np.asarray(k_b, np.float64)) / np.asarray(k_g, np.float64)),
        "tv_c": _const_or_none((THETA - np.asarray(v_b, np.float64)) / np.asarray(v_g, np.float64)),
        "tp_c": _const_or_none((THETA - np.asarray(proj_beta, np.float64)) / np.asarray(proj_g, np.float64)),
        "t1_c": _const_or_none((THETA - np.asarray(fc1_beta, np.float64)) / np.asarray(fc1_g, np.float64)),
        "t2_c": _const_or_none((THETA - np.asarray(fc2_beta, np.float64)) / np.asarray(fc2_g, np.float64)),
        "b1_sum": float(np.asarray(fc1_bias, np.float64).sum()),
        "mode_q": _sign_mode(np.asarray(q_g)), "mode_k": _sign_mode(np.asarray(k_g)),
        "mode_v": _sign_mode(np.asarray(v_g)), "mode_p": _sign_mode(np.asarray(proj_g)),
        "mode_1": _sign_mode(np.asarray(fc1_g)), "mode_2": _sign_mode(np.asarray(fc2_g)),
        "has_bp": bool(np.any(np.asarray(proj_bias) != 0)),
        "has_b1": bool(np.any(np.asarray(fc1_bias) != 0)),
        "has_b2": bool(np.any(np.asarray(fc2_bias) != 0)),
    }
    biases = {"bp": np.asarray(proj_bias, f32), "b1": np.asarray(fc1_bias, f32),
              "b2": np.asarray(fc2_bias, f32)}

    use_cc = cfg["use_cc"]
    in_maps = []
    for c in range(NCORES):
        b = c // 2
        h = c % 2
        own = X[b * L + h * T: b * L + (h + 1) * T]
        if use_cc:
            xT = np.ascontiguousarray(own.T)                # [D, T]
        else:
            other = X[b * L + (1 - h) * T: b * L + (2 - h) * T]
            Xp = np.concatenate([own, other], axis=0)       # [TB, D] own-first
            xT = np.ascontiguousarray(Xp.T)                 # [D, TB]
        xT_hi, xT_lo = _split_hi_lo(xT)
        m = {"xT_hi": xT_hi, "xT_lo": xT_lo,
             "x_tok": np.ascontiguousarray(own),
             "ident": np.eye(P, dtype=np.float32).astype(ml_dtypes.bfloat16),
             "ws1_hi": ws1_hi, "ws1_lo": ws1_lo}
        m.update(wsplit)
        m.update(thr)
        for nm in ("bp", "b1", "b2"):
            if cfg[f"has_{nm}"]:
                m[nm] = biases[nm]
        in_maps.append(m)
    return in_maps, cfg


_prog_cache = {}


def kernel(**inputs) -> np.ndarray:
    in_maps, cfg = make_core_inputs(**inputs)
    key = tuple(sorted(cfg.items()))
    if key not in _prog_cache:
        _prog_cache[key] = build_program(cfg)
    nc = _prog_cache[key]

    res = run_bass_kernel_spmd(nc, in_maps, core_ids=list(range(NCORES)))
    last_run_info["exec_time_ns"] = res.exec_time_ns
    last_run_info["mean_exec_time_ns"] = res.mean_exec_time_ns

    out = np.empty((B, L, D), np.float32)
    for c in range(NCORES):
        b = c // 2
        h = c % 2
        out[b, h * T:(h + 1) * T, :] = res.results[c]["out"]
    return out
